# revision 1
# baseline (speedup 1.0000x reference)
"""CARC attention processor kernel for 8 Trainium2 NeuronCores.

Reference computation (B=1, L=4096, C=640, H=10, D=64):
    q/k/v = hidden @ Wq/Wk/Wv, split into 10 heads of 64
    k_cat = [k, 0.42*K_bg], v_cat = [v, 0.42*V_bg]   (key length 8192)
    out   = softmax(q k_cat^T / 8) v_cat, heads merged, @ Wo + bo

Sharding: queries are split 512 per core; every core computes all 10 heads
for its queries (k/v projections replicated per core — cheap relative to
attention).  Output is a disjoint row-slice per core; the host concatenates.

All matmuls run in bf16 with fp32 PSUM accumulation.  Softmax skips the
max-subtraction (scores are ~N(0,1); exp runs on ScalarE with the 1/8 scale
folded in, and the 0.42 key-side scale folded into the bg exp scale).  The
softmax denominator comes from a ones-column appended to V in the probs@V
matmul; the output-projection bias is folded in as a 65th row of Wo against
the ctx ones-column.

Heads are processed in pairs: projections compute both heads of a pair in
one matmul stream (head A on partitions 0-63, head B on 64-127), exp reads
1024-wide (two PSUM banks) per instruction, and the A/B score tiles
alternate through a shared 2-slot PSUM pool so ScalarE (the critical
engine) never starves.
"""

import numpy as np

import concourse.bass as bass
import concourse.mybir as mybir
import concourse.tile as tile

F32 = mybir.dt.float32
BF16 = mybir.dt.bfloat16
AF = mybir.ActivationFunctionType

# Problem constants (hardcoded per contract)
B, L, C = 1, 4096, 640
H, D = 10, 64
ALPHA = 0.42
N_CORES = 8
SCALE = 1.0 / np.sqrt(D)  # 0.125


class Cfg:
    def __init__(self, H=H, C=C, Lk=L, Q=L // N_CORES):
        assert C % 128 == 0 and Lk % 1024 == 0 and Q % 128 == 0 and Q <= 512
        assert H % 2 == 0
        self.H, self.C, self.Lk, self.Q = H, C, Lk, Q
        self.n_cc = C // 128      # contraction chunks for projections
        self.n_kt = Lk // 128     # key tiles per source (self / bg)
        self.n_qt = Q // 128      # query tiles of this core


def emit(nc: bass.Bass, cfg: Cfg):
    Hh, Cc, Lk, Q = cfg.H, cfg.C, cfg.Lk, cfg.Q
    n_cc, n_kt, n_qt = cfg.n_cc, cfg.n_kt, cfg.n_qt
    n_pair = Hh // 2

    hT = nc.declare_dram_parameter("hT", [Cc, Lk], F32, isOutput=False)
    hqT = nc.declare_dram_parameter("hqT", [Cc, Q], F32, isOutput=False)
    kbgT = nc.declare_dram_parameter("KbgT", [Hh, D, Lk], F32, isOutput=False)
    vbg = nc.declare_dram_parameter("Vbg", [Hh, Lk, D], F32, isOutput=False)
    wq = nc.declare_dram_parameter("Wq", [Cc, Cc], F32, isOutput=False)
    wk = nc.declare_dram_parameter("Wk", [Cc, Cc], F32, isOutput=False)
    wv = nc.declare_dram_parameter("Wv", [Cc, Cc], F32, isOutput=False)
    wob = nc.declare_dram_parameter("WoB", [Hh, D + 1, Cc], F32, isOutput=False)
    out = nc.declare_dram_parameter("out", [Q, Cc], F32, isOutput=True)

    with tile.TileContext(nc) as tc:
        with (
            tc.tile_pool(name="singles", bufs=1) as singles,
            tc.tile_pool(name="stage", bufs=1) as stage,
            tc.tile_pool(name="bgstage", bufs=2) as bgstage,
            tc.tile_pool(name="kv", bufs=2) as kv,
            tc.tile_pool(name="probs", bufs=3) as probs_pool,
            tc.tile_pool(name="outsb", bufs=2) as outsb_pool,
            tc.tile_pool(name="fin", bufs=2) as fin_pool,
            tc.tile_pool(name="ps_a", bufs=2, space="PSUM") as ps_a,
            tc.tile_pool(name="ps_sc", bufs=2, space="PSUM") as ps_sc,
            tc.tile_pool(name="ps_ctx", bufs=1, space="PSUM") as ps_ctx,
        ):
            # ---- persistent SBUF tensors ----
            hT_bf = singles.tile([128, n_cc, Lk], BF16, tag="hT_bf")
            hq_bf = singles.tile([128, n_cc, Q], BF16, tag="hq_bf")
            wq_bf = singles.tile([128, n_cc, Cc], BF16, tag="wq_bf")
            wk_bf = singles.tile([128, n_cc, Cc], BF16, tag="wk_bf")
            wv_bf = singles.tile([128, n_cc, Cc], BF16, tag="wv_bf")
            wob_bf = singles.tile([D + 1, Hh, Cc], BF16, tag="wob_bf")
            qT2_all = singles.tile([128, n_pair, Q], BF16, tag="qT2_all")
            ctxT_all = singles.tile([D + 1, Hh, Q], BF16, tag="ctxT_all")
            ones64 = singles.tile([D + 1, D], F32, tag="ones64")
            nc.vector.memset(ones64, 1.0)

            # ---- load + cast hidden (transposed) and weights ----
            for i in range(n_cc):
                st = stage.tile([128, Lk], F32, tag="stage")
                nc.sync.dma_start(out=st, in_=hT[128 * i : 128 * (i + 1), :])
                nc.vector.tensor_copy(out=hT_bf[:, i, :], in_=st)
            for i in range(n_cc):
                st = stage.tile([128, Q], F32, tag="stage")
                nc.sync.dma_start(out=st, in_=hqT[128 * i : 128 * (i + 1), :])
                nc.vector.tensor_copy(out=hq_bf[:, i, :], in_=st)
            for w_dram, w_sb in ((wq, wq_bf), (wk, wk_bf), (wv, wv_bf)):
                st = stage.tile([128, n_cc, Cc], F32, tag="stage")
                nc.sync.dma_start(
                    out=st, in_=w_dram.rearrange("(i p) n -> p i n", p=128)
                )
                nc.vector.tensor_copy(out=w_sb, in_=st)
            hh = Hh // 2
            for half in range(2):
                st = stage.tile([D + 1, hh, Cc], F32, tag="stage")
                nc.sync.dma_start(
                    out=st,
                    in_=wob[half * hh : (half + 1) * hh].rearrange("h p n -> p h n"),
                )
                nc.vector.tensor_copy(
                    out=wob_bf[:, half * hh : (half + 1) * hh, :], in_=st
                )

            # ---- q projections, head pairs packed on partitions ----
            for p in range(n_pair):
                ps = ps_a.tile([128, Q], F32, tag="ps_a", name=f"qps{p}")
                for i in range(n_cc):
                    nc.tensor.matmul(
                        ps,
                        lhsT=wq_bf[:, i, 128 * p : 128 * (p + 1)],
                        rhs=hq_bf[:, i, :],
                        start=(i == 0),
                        stop=(i == n_cc - 1),
                    )
                nc.vector.tensor_copy(out=qT2_all[:, p, :], in_=ps)

            # ---- per head-pair: project k/v, load bg kv, attention ----
            for p in range(n_pair):
                kT2 = kv.tile([128, Lk], BF16, tag="kT")
                v2t = kv.tile([128, n_kt, 2 * (D + 1)], BF16, tag="v")
                kbg2 = kv.tile([128, Lk], BF16, tag="kbg")
                vbg2 = kv.tile([128, n_kt, 2 * (D + 1)], BF16, tag="vbg")

                # kT2 = (hidden @ Wk_pair)^T, head A on partitions 0-63
                for t in range(Lk // 512):
                    ps = ps_a.tile([128, 512], F32, tag="ps_a", name=f"kps{p}{t}")
                    for i in range(n_cc):
                        nc.tensor.matmul(
                            ps,
                            lhsT=wk_bf[:, i, 128 * p : 128 * (p + 1)],
                            rhs=hT_bf[:, i, 512 * t : 512 * (t + 1)],
                            start=(i == 0),
                            stop=(i == n_cc - 1),
                        )
                    nc.vector.tensor_copy(
                        out=kT2[:, 512 * t : 512 * (t + 1)], in_=ps
                    )
                # v natural [keys, D] for both heads (+ones cols)
                for kt in range(n_kt):
                    ps = ps_a.tile([128, 128], F32, tag="ps_a", name=f"vps{p}{kt}")
                    for i in range(n_cc):
                        nc.tensor.matmul(
                            ps,
                            lhsT=hT_bf[:, i, 128 * kt : 128 * (kt + 1)],
                            rhs=wv_bf[:, i, 128 * p : 128 * (p + 1)],
                            start=(i == 0),
                            stop=(i == n_cc - 1),
                        )
                    nc.vector.tensor_copy(out=v2t[:, kt, 0:D], in_=ps[:, 0:D])
                    nc.vector.tensor_copy(
                        out=v2t[:, kt, D + 1 : 2 * D + 1], in_=ps[:, D : 2 * D]
                    )
                nc.vector.memset(v2t[:, :, D : D + 1], 1.0)
                nc.vector.memset(v2t[:, :, 2 * D + 1 : 2 * D + 2], 1.0)

                # bg K (transposed) and bg V (scaled by ALPHA at load),
                # staged in 1/4 pieces to bound SBUF staging space
                for p4 in range(4):
                    lw = Lk // 4
                    tw = n_kt // 4
                    st = bgstage.tile([128, lw], F32, tag="kbg_st", name=f"kst{p}{p4}")
                    nc.sync.dma_start(
                        out=st[0:D, :], in_=kbgT[2 * p, :, lw * p4 : lw * (p4 + 1)]
                    )
                    nc.sync.dma_start(
                        out=st[D : 2 * D, :],
                        in_=kbgT[2 * p + 1, :, lw * p4 : lw * (p4 + 1)],
                    )
                    nc.vector.tensor_copy(
                        out=kbg2[:, lw * p4 : lw * (p4 + 1)], in_=st
                    )
                    st2 = bgstage.tile(
                        [128, tw, 2 * D], F32, tag="vbg_st", name=f"vst{p}{p4}"
                    )
                    for hi in range(2):
                        nc.sync.dma_start(
                            out=st2[:, :, D * hi : D * (hi + 1)],
                            in_=vbg[
                                2 * p + hi, lw * p4 : lw * (p4 + 1), :
                            ].rearrange("(kt q) d -> q kt d", q=128),
                        )
                        nc.vector.tensor_scalar_mul(
                            vbg2[
                                :,
                                tw * p4 : tw * (p4 + 1),
                                (D + 1) * hi : (D + 1) * hi + D,
                            ],
                            st2[:, :, D * hi : D * (hi + 1)],
                            ALPHA,
                        )
                nc.vector.memset(vbg2[:, :, D : D + 1], 1.0)
                nc.vector.memset(vbg2[:, :, 2 * D + 1 : 2 * D + 2], 1.0)

                # ---- attention for the pair ----
                # ctx accumulators: head A in PSUM bank 0, head B in bank 1
                ctx2 = ps_ctx.tile([D + 1, 2, 512], F32, tag="ctx", name=f"ctx{p}")
                n_k2 = n_kt // 2
                for src in range(2):  # 0=self keys, 1=bg keys
                    kk = kT2 if src == 0 else kbg2
                    vv = v2t if src == 0 else vbg2
                    e_scale = SCALE if src == 0 else SCALE * ALPHA
                    for k2 in range(n_k2):
                        first = src == 0 and k2 == 0
                        last = src == 1 and k2 == n_k2 - 1
                        # QK for heads A/B issued back-to-back per key tile:
                        # distinct PE row groups (tile_position) let the two
                        # K=64 matmuls stream concurrently.
                        scs = []
                        for hi in range(2):
                            scs.append(
                                ps_sc.tile(
                                    [128, 2, Q],
                                    F32,
                                    tag="sc",
                                    name=f"sc{p}{src}{k2}{hi}",
                                )
                            )
                        for j in range(2):
                            kt = 2 * k2 + j
                            for hi in range(2):
                                nc.tensor.matmul(
                                    scs[hi][:, j, :],
                                    lhsT=kk[
                                        D * hi : D * (hi + 1),
                                        128 * kt : 128 * (kt + 1),
                                    ],
                                    rhs=qT2_all[D * hi : D * (hi + 1), p, :],
                                    start=True,
                                    stop=True,
                                    tile_position=(D * hi, 0),
                                )
                        prs = []
                        for hi in range(2):
                            pr = probs_pool.tile(
                                [128, 2, Q], BF16, tag="pr", name=f"pr{p}{src}{k2}{hi}"
                            )
                            nc.scalar.activation(pr, scs[hi], AF.Exp, scale=e_scale)
                            prs.append(pr)
                        for hi in range(2):
                            for j in range(2):
                                kt = 2 * k2 + j
                                nc.tensor.matmul(
                                    ctx2[:, hi, 0:Q],
                                    lhsT=vv[
                                        :, kt, (D + 1) * hi : (D + 1) * (hi + 1)
                                    ],
                                    rhs=prs[hi][:, j, :],
                                    start=(first and j == 0),
                                    stop=(last and j == 1),
                                )
                # normalize: denom row (partition 64) -> broadcast over the
                # 64 d-partitions via a K=1 fp32 matmul, then recip + mul
                for hi in range(2):
                    h = 2 * p + hi
                    fin = fin_pool.tile([D + 1, Q], F32, tag="fin", name=f"fin{h}")
                    nc.vector.tensor_copy(
                        out=fin[D : D + 1, :], in_=ctx2[D : D + 1, hi, 0:Q]
                    )
                    bc = ps_a.tile([D, Q], F32, tag="ps_a", name=f"bc{h}")
                    nc.tensor.matmul(
                        bc,
                        lhsT=ones64[D : D + 1, :],
                        rhs=fin[D : D + 1, :],
                        start=True,
                        stop=True,
                        tile_position=(D, 0),
                    )
                    nc.vector.reciprocal(fin[0:D, :], bc)
                    nc.vector.tensor_mul(
                        ctxT_all[0:D, h, :], ctx2[0:D, hi, 0:Q], fin[0:D, :]
                    )
                    nc.vector.memset(ctxT_all[D : D + 1, h, :], 1.0)

            # ---- output projection: out[qt] = sum_h ctxT_h^T @ WoB_h ----
            for qt in range(n_qt):
                o_sb = outsb_pool.tile([128, Cc], F32, tag="o_sb")
                for n0 in range(0, Cc, 512):
                    nw = min(512, Cc - n0)
                    ps = ps_sc.tile([128, 2, Q], F32, tag="sc", name=f"ops{qt}{n0}")
                    for h in range(Hh):
                        nc.tensor.matmul(
                            ps[:, 0, 0:nw],
                            lhsT=ctxT_all[:, h, 128 * qt : 128 * (qt + 1)],
                            rhs=wob_bf[:, h, n0 : n0 + nw],
                            start=(h == 0),
                            stop=(h == Hh - 1),
                        )
                    nc.vector.tensor_copy(out=o_sb[:, n0 : n0 + nw], in_=ps[:, 0, 0:nw])
                nc.sync.dma_start(
                    out=out[128 * qt : 128 * (qt + 1), :], in_=o_sb
                )
    return nc


def split_waits(nc, limit=1):
    """This container's walrus rejects >limit sync waits per instruction;
    hoist excess waits onto standalone EventSemaphore instructions."""
    cnt = 0
    for f in nc.m.functions:
        for bb in f.blocks:
            fixed = []
            for inst in bb.instructions:
                si = inst.sync_info
                if si is not None and len(si.on_wait) > limit:
                    waits = list(si.on_wait)
                    extra, keep = waits[:-limit], waits[-limit:]
                    for w in extra:
                        cnt += 1
                        ev = mybir.InstEventSemaphore(
                            name=f"I-waitsplit-{cnt}", ins=[], outs=[]
                        )
                        ev.engine = inst.engine
                        ev.sync_info = mybir.SyncInfo(on_wait=[w], on_update=[])
                        nc.register_instruction(ev)
                        fixed.append(ev)
                    si.on_wait = keep
                fixed.append(inst)
            bb.instructions[:] = fixed
    return cnt


def build_bass(cfg: Cfg | None = None):
    cfg = cfg or Cfg()
    nc = bass.Bass()
    emit(nc, cfg)
    split_waits(nc)
    return nc


def make_in_maps(hidden_states, K_bg, V_bg, Wq, Wk, Wv, Wo, bo):
    hT = np.ascontiguousarray(np.asarray(hidden_states, np.float32)[0].T)
    KbgT = np.ascontiguousarray(np.asarray(K_bg, np.float32).transpose(0, 2, 1))
    WoB = np.zeros((H, D + 1, C), np.float32)
    WoB[:, :D, :] = np.asarray(Wo, np.float32).reshape(H, D, C)
    WoB[0, D, :] = np.asarray(bo, np.float32)
    common = {
        "hT": hT,
        "KbgT": KbgT,
        "Vbg": np.ascontiguousarray(np.asarray(V_bg, np.float32)),
        "Wq": np.asarray(Wq, np.float32),
        "Wk": np.asarray(Wk, np.float32),
        "Wv": np.asarray(Wv, np.float32),
        "WoB": WoB,
    }
    qs = L // N_CORES
    return [
        dict(common, hqT=np.ascontiguousarray(hT[:, qs * c : qs * (c + 1)]))
        for c in range(N_CORES)
    ]


_NC_CACHE = {}


def kernel(hidden_states, K_bg, V_bg, Wq, Wk, Wv, Wo, bo):
    if "nc" not in _NC_CACHE:
        _NC_CACHE["nc"] = build_bass()
    nc = _NC_CACHE["nc"]
    in_maps = make_in_maps(hidden_states, K_bg, V_bg, Wq, Wk, Wv, Wo, bo)
    from concourse import bass2jax

    results = bass2jax.run_bass_via_pjrt(nc, in_maps, n_cores=N_CORES)
    out = np.concatenate([results[c]["out"] for c in range(N_CORES)], axis=0)
    return out.reshape(B, L, C)



# revision 11
# speedup vs baseline: 1.2674x; 1.2674x over previous
"""CARC attention processor kernel for 8 Trainium2 NeuronCores — v2.

Reference computation (B=1, L=4096, C=640, H=10, D=64):
    q/k/v = hidden @ Wq/Wk/Wv, split into 10 heads of 64
    k_cat = [k, 0.42*K_bg], v_cat = [v, 0.42*V_bg]   (key length 8192)
    out   = softmax(q k_cat^T / 8) v_cat, heads merged, @ Wo + bo

Sharding: queries split 512 per core; every core computes all 10 heads for
its queries (k/v projections replicated per core).

v2 design (vs baseline): the ScalarE exp (41.9M elements/core, ~(N+352)/1.2
ns per ACTIVATE) is the hard floor; everything else is organized to keep
ScalarE ~100% busy on exp with every other engine underneath it:
  - all-fp16 data path; the host pre-casts/pre-arranges inputs so no
    on-chip input casts are needed and DMA descriptors are >=1KB.
  - same-head QK pairing: q is duplicated onto both partition halves (via
    col-tiled projection matmuls, free), and kT holds even key tiles on
    partitions 0:64 / odd on 64:128 (K-proj col-tiled via tile_position).
    One [128,2,512] PSUM tile = 2 key tiles of ONE head -> exp N=1024,
    double-buffered in 4 PSUM banks, leaving ctx 2 banks + proj 2 banks.
  - V projected 8+2 heads per fill (512/128-wide streams, shared
    LDWEIGHTS) into a persistent v2 table [128, 32, 10, 65] with ones
    columns (softmax denominator = ctx row 64).  V_bg arrives pre-
    interleaved with ones baked and ALPHA folded; K_bg pre-split/scaled.
  - bg-key attention first (no projection dependency) hides the prologue;
    projections are a deferred work-queue drained between supersteps so
    the PE never idles long enough for the HAM clock gate to re-throttle.
"""

from collections import deque

import numpy as np

import concourse.bass as bass
import concourse.mybir as mybir
import concourse.tile as tile

F32 = mybir.dt.float32
F16 = mybir.dt.float16
AF = mybir.ActivationFunctionType

B, L, C = 1, 4096, 640
H, D = 10, 64
ALPHA = 0.42
N_CORES = 8
SCALE = 1.0 / np.sqrt(D)  # 0.125
Q = L // N_CORES  # 512
NKT = L // 128  # 32 key tiles per source
NCC = C // 128  # 5 contraction chunks


def emit(nc: bass.Bass):
    n_pair = H // 2

    hT = nc.declare_dram_parameter("hT", [NCC, 128, L], F16, isOutput=False)
    hq = nc.declare_dram_parameter("hq", [NCC, 128, Q], F16, isOutput=False)
    wq = nc.declare_dram_parameter("wq", [NCC, 128, C], F16, isOutput=False)
    wk = nc.declare_dram_parameter("wk", [NCC, 128, C], F16, isOutput=False)
    wv = nc.declare_dram_parameter("wv", [NCC, 128, C], F16, isOutput=False)
    wob = nc.declare_dram_parameter("wob", [H, D + 1, C], F16, isOutput=False)
    kbgS = nc.declare_dram_parameter("kbgS", [H, 128, L // 2], F16, isOutput=False)
    vbgS = nc.declare_dram_parameter(
        "vbgS", [n_pair, 128, NKT * 130], F16, isOutput=False
    )
    out = nc.declare_dram_parameter("out", [Q, C], F32, isOutput=True)

    with tile.TileContext(nc) as tc:
        with (
            tc.tile_pool(name="singles", bufs=1) as singles,
            tc.tile_pool(name="kbgp", bufs=2) as kbgp,
            tc.tile_pool(name="vbgp", bufs=2) as vbgp,
            tc.tile_pool(name="ktp", bufs=2) as ktp,
            tc.tile_pool(name="probs", bufs=3) as probs_pool,
            tc.tile_pool(name="fin", bufs=2) as fin_pool,
            tc.tile_pool(name="outsb", bufs=1) as outsb_pool,
            tc.tile_pool(name="ps_sc", bufs=2, space="PSUM") as ps_sc,
            tc.tile_pool(name="ps_ctx", bufs=1, space="PSUM") as ps_ctx,
            tc.tile_pool(name="ps_pj", bufs=2, space="PSUM") as ps_pj,
        ):
            # ---- persistent SBUF ----
            hT_sb = singles.tile([128, NCC, L], F16, tag="hT")
            hq_sb = singles.tile([128, NCC, Q], F16, tag="hq")
            wq_sb = singles.tile([128, NCC, C], F16, tag="wq")
            wk_sb = singles.tile([128, NCC, C], F16, tag="wk")
            wv_sb = singles.tile([128, NCC, C], F16, tag="wv")
            wob_sb = singles.tile([D + 1, H, C], F16, tag="wob")
            qdup = singles.tile([128, H, Q], F16, tag="qdup")
            v2 = singles.tile([128, NKT, H, D + 1], F16, tag="v2")
            ctxT = singles.tile([D + 1, H, Q], F16, tag="ctxT")
            ones64 = singles.tile([D + 1, D], F32, tag="ones64")
            nc.vector.memset(ones64, 1.0)
            nc.vector.memset(v2[:, :, :, D : D + 1], 1.0)

            # ---- input DMAs, ordered so pair-0 bg attention starts early
            nc.sync.dma_start(out=hq_sb, in_=hq.rearrange("i p n -> p i n"))
            nc.sync.dma_start(out=wq_sb, in_=wq.rearrange("i p n -> p i n"))
            kbg_t = {}
            vbg_t = {}

            def stage_bg(p):
                kbg_t[p] = kbgp.tile([128, 2, L // 2], F16, tag="kbg", name=f"kbg{p}")
                for hi in range(2):
                    nc.sync.dma_start(out=kbg_t[p][:, hi, :], in_=kbgS[2 * p + hi, :, :])
                vbg_t[p] = vbgp.tile([128, NKT, 130], F16, tag="vbg", name=f"vbg{p}")
                nc.sync.dma_start(
                    out=vbg_t[p].rearrange("p t c -> p (t c)"), in_=vbgS[p]
                )

            stage_bg(0)
            stage_bg(1)
            nc.sync.dma_start(out=wk_sb, in_=wk.rearrange("i p n -> p i n"))
            nc.sync.dma_start(out=hT_sb, in_=hT.rearrange("i p n -> p i n"))
            nc.sync.dma_start(out=wv_sb, in_=wv.rearrange("i p n -> p i n"))
            nc.sync.dma_start(out=wob_sb, in_=wob.rearrange("h p n -> p h n"))

            # ---------- projection work items ----------
            kT_t = {}

            def do_qproj(h):
                # q for head h duplicated onto both partition halves via
                # col-tiled matmuls sharing one LDWEIGHTS per chunk
                ps = ps_pj.tile([128, Q], F32, tag="pj", name=f"qps{h}")
                for i in range(NCC):
                    for par in range(2):
                        nc.tensor.matmul(
                            ps[64 * par : 64 * (par + 1), :],
                            lhsT=wq_sb[:, i, 64 * h : 64 * (h + 1)],
                            rhs=hq_sb[:, i, :],
                            start=(i == 0),
                            stop=(i == NCC - 1),
                            tile_position=(0, 64 * par),
                            skip_group_check=True,
                        )
                nc.vector.tensor_copy(out=qdup[:, h, :], in_=ps)

            def do_kproj(p, hi, f):
                # fill f covers keys [1024f, 1024(f+1)): even key tiles on
                # partitions 0:64, odd on 64:128 (col-tiled, interleaved
                # per chunk so the two groups run concurrently)
                h = 2 * p + hi
                ps = ps_pj.tile([128, Q], F32, tag="pj", name=f"kps{p}{hi}{f}")
                hT_blk = hT_sb[:, :, 1024 * f : 1024 * (f + 1)].rearrange(
                    "p i (a b n) -> p i a b n", b=2, n=128
                )
                for i in range(NCC):
                    for par in range(2):
                        nc.tensor.matmul(
                            ps[64 * par : 64 * (par + 1), :],
                            lhsT=wk_sb[:, i, 64 * h : 64 * (h + 1)],
                            rhs=hT_blk[:, i, :, par, :],
                            start=(i == 0),
                            stop=(i == NCC - 1),
                            tile_position=(0, 64 * par),
                            skip_group_check=True,
                        )
                nc.vector.tensor_copy(out=kT_t[p][:, hi, f, :], in_=ps)

            def do_vproj(t):
                # v for all 10 heads of key tile t: one 512-wide fill
                # (heads 0-7) + one 128-wide fill (heads 8-9)
                psA = ps_pj.tile([128, 512], F32, tag="pj", name=f"vpsA{t}")
                for i in range(NCC):
                    nc.tensor.matmul(
                        psA,
                        lhsT=hT_sb[:, i, 128 * t : 128 * (t + 1)],
                        rhs=wv_sb[:, i, 0:512],
                        start=(i == 0),
                        stop=(i == NCC - 1),
                    )
                nc.vector.tensor_copy(
                    out=v2[:, t, 0:8, 0:D],
                    in_=psA.rearrange("p (h d) -> p h d", d=64),
                )
                psB = ps_pj.tile([128, 128], F32, tag="pj", name=f"vpsB{t}")
                for i in range(NCC):
                    nc.tensor.matmul(
                        psB,
                        lhsT=hT_sb[:, i, 128 * t : 128 * (t + 1)],
                        rhs=wv_sb[:, i, 512:640],
                        start=(i == 0),
                        stop=(i == NCC - 1),
                    )
                nc.vector.tensor_copy(
                    out=v2[:, t, 8:10, 0:D],
                    in_=psB.rearrange("p (h d) -> p h d", d=64),
                )

            proj_work = deque()
            proj_done = set()

            def run_item(item):
                if item[0] == "q":
                    do_qproj(item[1])
                elif item[0] == "k":
                    do_kproj(item[1], item[2], item[3])
                else:
                    do_vproj(item[1])
                proj_done.add(item)

            def drain(n=1):
                for _ in range(n):
                    if proj_work:
                        run_item(proj_work.popleft())

            def ensure(item):
                while item not in proj_done:
                    assert proj_work, f"work item {item} never queued"
                    run_item(proj_work.popleft())

            def queue_kproj(p):
                kT_t[p] = ktp.tile([128, 2, 4, Q], F16, tag="kT", name=f"kT{p}")
                for hi in range(2):
                    for f in range(4):
                        proj_work.append(("k", p, hi, f))

            # prologue: q heads 0/1 inline (first bg supersteps need them)
            do_qproj(0)
            do_qproj(1)
            proj_done.add(("q", 0))
            proj_done.add(("q", 1))
            for h in range(2, H):
                proj_work.append(("q", h))
            queue_kproj(0)
            for t in range(NKT):
                proj_work.append(("v", t))
            queue_kproj(1)

            # ---- attention: per pair, bg phase then self phase ----
            for p in range(n_pair):
                ctx2 = ps_ctx.tile([D + 1, 2, Q], F32, tag="ctx", name=f"ctx{p}")
                if p + 2 < n_pair:
                    stage_bg(p + 2)
                if p + 1 < n_pair and p + 1 not in kT_t:
                    queue_kproj(p + 1)

                pend = deque()  # PV work pending behind QK+exp

                def superstep(kind, ss, hi, p=p):
                    h = 2 * p + hi
                    if kind == "bg":
                        klhs = lambda par: kbg_t[p][
                            64 * par : 64 * (par + 1), hi, 128 * ss : 128 * (ss + 1)
                        ]
                    else:
                        f, c4 = ss // 4, ss % 4
                        klhs = lambda par: kT_t[p][
                            64 * par : 64 * (par + 1), hi, f, 128 * c4 : 128 * (c4 + 1)
                        ]
                    sc = ps_sc.tile(
                        [128, 2, Q], F32, tag="sc", name=f"sc{kind}{p}{ss}{hi}"
                    )
                    for par in range(2):
                        nc.tensor.matmul(
                            sc[:, par, :],
                            lhsT=klhs(par),
                            rhs=qdup[64 * par : 64 * (par + 1), h, :],
                            start=True,
                            stop=True,
                            tile_position=(64 * par, 0),
                        )
                    pr = probs_pool.tile(
                        [128, 2, Q], F16, tag="pr", name=f"pr{kind}{p}{ss}{hi}"
                    )
                    nc.scalar.activation(pr, sc, AF.Exp, scale=SCALE)
                    return pr

                def do_pv(kind, ss, hi, pr, p=p, ctx2=ctx2):
                    h = 2 * p + hi
                    for j in range(2):
                        t = 2 * ss + j
                        if kind == "bg":
                            vlhs = vbg_t[p][:, t, 65 * hi : 65 * (hi + 1)]
                        else:
                            ensure(("v", t))
                            vlhs = v2[:, t, h, :]
                        first = kind == "bg" and ss == 0 and j == 0
                        last = kind == "self" and ss == 15 and j == 1
                        nc.tensor.matmul(
                            ctx2[:, hi, :],
                            lhsT=vlhs,
                            rhs=pr[:, j, :],
                            start=first,
                            stop=last,
                        )

                steps = [("bg", ss, hi) for ss in range(16) for hi in range(2)]
                steps += [("self", ss, hi) for ss in range(16) for hi in range(2)]
                for kind, ss, hi in steps:
                    if kind == "bg" and ss == 0:
                        ensure(("q", 2 * p + hi))
                    if kind == "self" and ss == 0:
                        for f in range(4):
                            ensure(("k", p, hi, f))
                    pr = superstep(kind, ss, hi)
                    while len(pend) >= 1:
                        do_pv(*pend.popleft())
                        drain(1)
                    pend.append((kind, ss, hi, pr))
                while pend:
                    do_pv(*pend.popleft())

                # copy unnormalized ctx out of PSUM fast (frees the ctx
                # banks so the next pair's PV isn't blocked), then
                # normalize from SBUF at leisure.
                ctxU = fin_pool.tile([D + 1, 2, Q], F32, tag="fin", name=f"cu{p}")
                nc.vector.tensor_copy(out=ctxU, in_=ctx2)
                for hi in range(2):
                    h = 2 * p + hi
                    bc = ps_pj.tile([D, Q], F32, tag="pj", name=f"bc{h}")
                    nc.tensor.matmul(
                        bc,
                        lhsT=ones64[D : D + 1, :],
                        rhs=ctxU[D : D + 1, hi, :],
                        start=True,
                        stop=True,
                        tile_position=(D, 0),
                    )
                    rec = fin_pool.tile([D, Q], F32, tag="rec", name=f"rec{h}")
                    nc.vector.reciprocal(rec, bc)
                    nc.vector.tensor_mul(
                        ctxT[0:D, h, :], ctxU[0:D, hi, :], rec
                    )
                    nc.vector.memset(ctxT[D : D + 1, h, :], 1.0)

            drain(len(proj_work))

            # ---- output projection ----
            for qt in range(Q // 128):
                o_sb = outsb_pool.tile([128, C], F32, tag="o_sb")
                for n0 in range(0, C, 512):
                    nw = min(512, C - n0)
                    ps = ps_sc.tile([128, 2, Q], F32, tag="sc", name=f"ops{qt}{n0}")
                    for h in range(H):
                        nc.tensor.matmul(
                            ps[:, 0, 0:nw],
                            lhsT=ctxT[:, h, 128 * qt : 128 * (qt + 1)],
                            rhs=wob_sb[:, h, n0 : n0 + nw],
                            start=(h == 0),
                            stop=(h == H - 1),
                        )
                    nc.vector.tensor_copy(
                        out=o_sb[:, n0 : n0 + nw], in_=ps[:, 0, 0:nw]
                    )
                nc.sync.dma_start(out=out[128 * qt : 128 * (qt + 1), :], in_=o_sb)
    return nc


def split_waits(nc, limit=1):
    """Hoist excess sync waits onto standalone EventSemaphore instructions."""
    cnt = 0
    for f in nc.m.functions:
        for bb in f.blocks:
            fixed = []
            for inst in bb.instructions:
                si = inst.sync_info
                if si is not None and len(si.on_wait) > limit:
                    waits = list(si.on_wait)
                    extra, keep = waits[:-limit], waits[-limit:]
                    for w in extra:
                        cnt += 1
                        ev = mybir.InstEventSemaphore(
                            name=f"I-waitsplit-{cnt}", ins=[], outs=[]
                        )
                        ev.engine = inst.engine
                        ev.sync_info = mybir.SyncInfo(on_wait=[w], on_update=[])
                        nc.register_instruction(ev)
                        fixed.append(ev)
                    si.on_wait = keep
                fixed.append(inst)
            bb.instructions[:] = fixed
    return cnt


def build_bass():
    nc = bass.Bass()
    emit(nc)
    split_waits(nc)
    return nc


def make_in_maps(hidden_states, K_bg, V_bg, Wq, Wk, Wv, Wo, bo):
    f16 = np.float16
    hidden = np.asarray(hidden_states, np.float32)[0]  # [L, C]
    hT5 = np.ascontiguousarray(hidden.T.reshape(NCC, 128, L)).astype(f16)

    def chunk_w(W):
        return np.ascontiguousarray(
            np.asarray(W, np.float32).reshape(NCC, 128, C)
        ).astype(f16)

    wq5, wk5, wv5 = chunk_w(Wq), chunk_w(Wk), chunk_w(Wv)

    WoB = np.zeros((H, D + 1, C), np.float32)
    WoB[:, :D, :] = np.asarray(Wo, np.float32).reshape(H, D, C)
    WoB[0, D, :] = np.asarray(bo, np.float32)

    # bg K: [H, 128, L/2], alpha folded, even key tiles on rows 0:64
    KbgT = np.asarray(K_bg, np.float32).transpose(0, 2, 1) * ALPHA  # [H, D, L]
    kv = KbgT.reshape(H, D, NKT, 128)
    kbgS = np.empty((H, 128, L // 2), np.float32)
    kbgS[:, 0:D, :] = kv[:, :, 0::2, :].reshape(H, D, L // 2)
    kbgS[:, D:128, :] = kv[:, :, 1::2, :].reshape(H, D, L // 2)
    kbgS = np.ascontiguousarray(kbgS).astype(f16)

    # bg V: [n_pair, 128, NKT*130], alpha folded, ones baked at cols 64/129
    Vb = (np.asarray(V_bg, np.float32) * ALPHA).reshape(H, NKT, 128, D)
    vbgS = np.ones((H // 2, 128, NKT, 130), np.float32)
    for hi in range(2):
        # [pairs, t, k, d] -> [pairs, k, t, d]
        vbgS[:, :, :, 65 * hi : 65 * hi + D] = Vb[hi::2].transpose(0, 2, 1, 3)
    vbgS = np.ascontiguousarray(vbgS.reshape(H // 2, 128, NKT * 130)).astype(f16)

    common = {
        "hT": hT5,
        "wq": wq5,
        "wk": wk5,
        "wv": wv5,
        "wob": WoB.astype(f16),
        "kbgS": kbgS,
        "vbgS": vbgS,
    }
    return [
        dict(common, hq=np.ascontiguousarray(hT5[:, :, Q * c : Q * (c + 1)]))
        for c in range(N_CORES)
    ]


_NC_CACHE = {}


def kernel(hidden_states, K_bg, V_bg, Wq, Wk, Wv, Wo, bo):
    if "nc" not in _NC_CACHE:
        _NC_CACHE["nc"] = build_bass()
    nc = _NC_CACHE["nc"]
    in_maps = make_in_maps(hidden_states, K_bg, V_bg, Wq, Wk, Wv, Wo, bo)
    from concourse import bass2jax

    results = bass2jax.run_bass_via_pjrt(nc, in_maps, n_cores=N_CORES)
    out = np.concatenate([results[c]["out"] for c in range(N_CORES)], axis=0)
    return out.reshape(B, L, C)


# revision 14
# speedup vs baseline: 1.3374x; 1.0552x over previous
"""CARC attention processor kernel for 8 Trainium2 NeuronCores — v3.

Reference computation (B=1, L=4096, C=640, H=10, D=64):
    q/k/v = hidden @ Wq/Wk/Wv, split into 10 heads of 64
    k_cat = [k, 0.42*K_bg], v_cat = [v, 0.42*V_bg]   (key length 8192)
    out   = softmax(q k_cat^T / 8) v_cat, heads merged, @ Wo + bo

Sharding: queries split 512 per core; every core computes all 10 heads for
its queries (k/v projections replicated per core).

The ScalarE exp (41.9M elements/core at ~(N+352)/1.2 ns per ACTIVATE,
N=1024) is the hard floor (~356us); the kernel keeps ScalarE saturated:
  - all-fp16 data path, host pre-arranged inputs, descriptor-friendly DMA.
  - same-head QK pairing (q duplicated onto both partition halves via
    col-tiled projection; kT split even/odd key tiles across partition
    halves) so one [128,2,512] PSUM tile = 2 key tiles of one head ->
    exp N=1024, double-buffered in 4 banks; ctx 2 banks; proj 2 banks.
  - every non-attention PE task (q/k/v projections, softmax-denominator
    broadcast, normalization, output projection) is a queue of micro work
    items (<=2 matmul-chunks each) drained between supersteps, with
    deadline ensure()s — so the PE FIFO never parks a long fill in front
    of the next score matmul and the HAM clock gate stays warm.
  - v-projection is split by head-group (0-3 / 4-7 / 8-9) so its deadline
    spreads across pairs instead of all landing in pair 0.
  - normalization: ctx leaves PSUM immediately (one DVE copy); the
    denominator broadcast / reciprocal_approx_fast / multiply run as
    deferred items inside the next pair.
  - output projection accumulates per-pair partial products into an SBUF
    fp16 accumulator (items), so the epilogue is just the last pair's
    items + one DMA; the f16->f32 output cast happens on the host.
"""

from collections import deque

import numpy as np

import concourse.bass as bass
import concourse.mybir as mybir
import concourse.tile as tile

F32 = mybir.dt.float32
F16 = mybir.dt.float16
AF = mybir.ActivationFunctionType

B, L, C = 1, 4096, 640
H, D = 10, 64
ALPHA = 0.42
N_CORES = 8
SCALE = 1.0 / np.sqrt(D)  # 0.125
Q = L // N_CORES  # 512
NKT = L // 128  # 32 key tiles per source
NCC = C // 128  # 5 contraction chunks
VGROUPS = ((0, 4), (4, 8), (8, 10))  # v-projection head groups


def emit(nc: bass.Bass):
    n_pair = H // 2

    hT = nc.declare_dram_parameter("hT", [NCC, 128, L], F16, isOutput=False)
    hq = nc.declare_dram_parameter("hq", [NCC, 128, Q], F16, isOutput=False)
    wq = nc.declare_dram_parameter("wq", [NCC, 128, C], F16, isOutput=False)
    wk = nc.declare_dram_parameter("wk", [NCC, 128, C], F16, isOutput=False)
    wv = nc.declare_dram_parameter("wv", [NCC, 128, C], F16, isOutput=False)
    wob = nc.declare_dram_parameter("wob", [H, D + 1, C], F16, isOutput=False)
    kbgS = nc.declare_dram_parameter("kbgS", [H, 128, L // 2], F16, isOutput=False)
    vbgS = nc.declare_dram_parameter(
        "vbgS", [n_pair, 128, NKT * 130], F16, isOutput=False
    )
    out = nc.declare_dram_parameter("out", [Q, C], F16, isOutput=True)

    with tile.TileContext(nc) as tc:
        with (
            tc.tile_pool(name="singles", bufs=1) as singles,
            tc.tile_pool(name="kbgp", bufs=2) as kbgp,
            tc.tile_pool(name="vbgp", bufs=2) as vbgp,
            tc.tile_pool(name="ktp", bufs=2) as ktp,
            tc.tile_pool(name="probs", bufs=3) as probs_pool,
            tc.tile_pool(name="fin", bufs=2) as fin_pool,
            tc.tile_pool(name="ps_sc", bufs=2, space="PSUM") as ps_sc,
            tc.tile_pool(name="ps_ctx", bufs=1, space="PSUM") as ps_ctx,
            tc.tile_pool(name="ps_pj", bufs=2, space="PSUM") as ps_pj,
        ):
            # ---- persistent SBUF ----
            hT_sb = singles.tile([128, NCC, L], F16, tag="hT")
            hq_sb = singles.tile([128, NCC, Q], F16, tag="hq")
            wq_sb = singles.tile([128, NCC, C], F16, tag="wq")
            wk_sb = singles.tile([128, NCC, C], F16, tag="wk")
            wv_sb = singles.tile([128, NCC, C], F16, tag="wv")
            wob_sb = singles.tile([D + 1, H, C], F16, tag="wob")
            qdup = singles.tile([128, H, Q], F16, tag="qdup")
            v2 = singles.tile([128, NKT, H, D + 1], F16, tag="v2")
            oacc = singles.tile([128, Q // 128, C], F16, tag="oacc")
            ones64 = singles.tile([D + 1, D], F16, tag="ones64")
            nc.vector.memset(ones64, 1.0)
            nc.vector.memset(v2[:, :, :, D : D + 1], 1.0)

            # ---- input DMAs ordered by first-use deadline ----
            nc.sync.dma_start(out=hq_sb, in_=hq.rearrange("i p n -> p i n"))
            nc.sync.dma_start(out=wq_sb, in_=wq.rearrange("i p n -> p i n"))
            kbg_t = {}
            vbg_t = {}

            def stage_bg(p):
                kbg_t[p] = kbgp.tile([128, 2, L // 2], F16, tag="kbg", name=f"kbg{p}")
                for hi in range(2):
                    nc.sync.dma_start(
                        out=kbg_t[p][:, hi, :], in_=kbgS[2 * p + hi, :, :]
                    )
                vbg_t[p] = vbgp.tile([128, NKT, 130], F16, tag="vbg", name=f"vbg{p}")
                nc.sync.dma_start(
                    out=vbg_t[p].rearrange("p t c -> p (t c)"), in_=vbgS[p]
                )

            stage_bg(0)
            nc.sync.dma_start(out=wk_sb, in_=wk.rearrange("i p n -> p i n"))
            nc.sync.dma_start(out=hT_sb, in_=hT.rearrange("i p n -> p i n"))
            nc.sync.dma_start(out=wv_sb, in_=wv.rearrange("i p n -> p i n"))
            nc.sync.dma_start(out=wob_sb, in_=wob.rearrange("h p n -> p h n"))
            stage_bg(1)

            # ---------- deferred micro work items ----------
            kT_t = {}
            psum_live = {}  # fill key -> live psum tile
            sbuf_live = {}  # ctxU / rec / ctxT tiles per pair
            work = deque()
            done = set()

            def run_item(item):
                key, fn = item
                fn()
                done.add(key)

            def drain(n=1):
                for _ in range(n):
                    if work:
                        run_item(work.popleft())

            def ensure(key):
                while key not in done:
                    assert work, f"work item {key} never queued"
                    run_item(work.popleft())

            # --- q projection: head h duplicated via col-tiled matmuls
            def q_items(h):
                def fill(i0, i1, h=h):
                    if i0 == 0:
                        psum_live["q", h] = ps_pj.tile(
                            [128, Q], F32, tag="pj", name=f"qps{h}"
                        )
                    ps = psum_live["q", h]
                    for i in range(i0, i1):
                        for par in range(2):
                            nc.tensor.matmul(
                                ps[64 * par : 64 * (par + 1), :],
                                lhsT=wq_sb[:, i, 64 * h : 64 * (h + 1)],
                                rhs=hq_sb[:, i, :],
                                start=(i == 0),
                                stop=(i == NCC - 1),
                                tile_position=(0, 64 * par),
                                skip_group_check=True,
                            )

                def copy(h=h):
                    nc.vector.tensor_copy(
                        out=qdup[:, h, :], in_=psum_live.pop(("q", h))
                    )

                return [
                    (("qf", h, 0), lambda h=h: fill(0, 3)),
                    (("qf", h, 1), lambda h=h: fill(3, NCC)),
                    (("q", h), copy),
                ]

            # --- k projection: pair p, head-in-pair hi, fill f (1024 keys,
            # even tiles -> partitions 0:64, odd -> 64:128)
            def k_items(p, hi, f):
                h = 2 * p + hi

                def chunk(i0, i1, p=p, hi=hi, f=f, h=h):
                    if i0 == 0:
                        psum_live["k", p, hi, f] = ps_pj.tile(
                            [128, Q], F32, tag="pj", name=f"kps{p}{hi}{f}"
                        )
                    ps = psum_live["k", p, hi, f]
                    hT_blk = hT_sb[:, :, 1024 * f : 1024 * (f + 1)].rearrange(
                        "p i (a b n) -> p i a b n", b=2, n=128
                    )
                    for i in range(i0, i1):
                        for par in range(2):
                            nc.tensor.matmul(
                                ps[64 * par : 64 * (par + 1), :],
                                lhsT=wk_sb[:, i, 64 * h : 64 * (h + 1)],
                                rhs=hT_blk[:, i, :, par, :],
                                start=(i == 0),
                                stop=(i == NCC - 1),
                                tile_position=(0, 64 * par),
                                skip_group_check=True,
                            )

                def copy(p=p, hi=hi, f=f):
                    nc.vector.tensor_copy(
                        out=kT_t[p][:, hi, f, :], in_=psum_live.pop(("k", p, hi, f))
                    )

                return [
                    (("kf", p, hi, f, 0), lambda p=p, hi=hi, f=f: chunk(0, 2)),
                    (("kf", p, hi, f, 1), lambda p=p, hi=hi, f=f: chunk(2, 4)),
                    (("kf", p, hi, f, 2), lambda p=p, hi=hi, f=f: chunk(4, NCC)),
                    (("k", p, hi, f), copy),
                ]

            def queue_kproj(p):
                kT_t[p] = ktp.tile([128, 2, 4, Q], F16, tag="kT", name=f"kT{p}")
                for hi in range(2):
                    for f in range(4):
                        work.extend(k_items(p, hi, f))

            # --- v projection for head group g, key tile t
            def v_items(g, t):
                h0, h1 = VGROUPS[g]
                nh = h1 - h0

                def chunk(i0, i1, g=g, t=t, h0=h0, nh=nh):
                    if i0 == 0:
                        psum_live["v", g, t] = ps_pj.tile(
                            [128, Q], F32, tag="pj", name=f"vps{g}_{t}"
                        )
                    ps = psum_live["v", g, t]
                    for i in range(i0, i1):
                        nc.tensor.matmul(
                            ps[:, 0 : 64 * nh],
                            lhsT=hT_sb[:, i, 128 * t : 128 * (t + 1)],
                            rhs=wv_sb[:, i, 64 * h0 : 64 * (h0 + nh)],
                            start=(i == 0),
                            stop=(i == NCC - 1),
                        )

                def copy(g=g, t=t, h0=h0, nh=nh):
                    ps = psum_live.pop(("v", g, t))
                    nc.vector.tensor_copy(
                        out=v2[:, t, h0 : h0 + nh, 0:D],
                        in_=ps[:, 0 : 64 * nh].rearrange("p (h d) -> p h d", d=64),
                    )

                return [
                    (("vf", g, t, 0), lambda g=g, t=t: chunk(0, 2)),
                    (("vf", g, t, 1), lambda g=g, t=t: chunk(2, 4)),
                    (("vf", g, t, 2), lambda g=g, t=t: chunk(4, NCC)),
                    (("v", g, t), copy),
                ]

            # --- normalization items for pair p (after ctxU copied to SBUF)
            def n_items(p):
                items = []
                ctxT = fin_pool.tile(
                    [D + 1, 2, Q], F16, tag="ctxT", name=f"ctxT{p}", bufs=2
                )
                sbuf_live["ctxT", p] = ctxT
                for hi in range(2):

                    def bc_fn(p=p, hi=hi):
                        ctxU = sbuf_live["ctxU", p]
                        bc = ps_pj.tile([D, Q], F32, tag="pj", name=f"bc{p}{hi}")
                        psum_live["bc", p, hi] = bc
                        nc.tensor.matmul(
                            bc,
                            lhsT=ones64[D : D + 1, :],
                            rhs=ctxU[D : D + 1, hi, :],
                            start=True,
                            stop=True,
                            tile_position=(D, 0),
                        )

                    def bcs_fn(p=p, hi=hi):
                        bcs = fin_pool.tile(
                            [D, Q], F32, tag="bcs", name=f"bcs{p}{hi}", bufs=2
                        )
                        sbuf_live["bcs", p, hi] = bcs
                        nc.vector.tensor_copy(
                            out=bcs, in_=psum_live.pop(("bc", p, hi))
                        )

                    def rec_fn(p=p, hi=hi):
                        rec = fin_pool.tile(
                            [D, Q], F32, tag="rec", name=f"rec{p}{hi}", bufs=2
                        )
                        sbuf_live["rec", p, hi] = rec
                        nc.vector.reciprocal(rec, sbuf_live.pop(("bcs", p, hi)))

                    def mul_fn(p=p, hi=hi):
                        ctxU = sbuf_live["ctxU", p]
                        ctxT_ = sbuf_live["ctxT", p]
                        nc.vector.tensor_mul(
                            ctxT_[0:D, hi, :],
                            ctxU[0:D, hi, :],
                            sbuf_live.pop(("rec", p, hi)),
                        )
                        nc.vector.memset(ctxT_[D : D + 1, hi, :], 1.0)

                    items += [
                        (("nb", p, hi), bc_fn),
                        (("nc", p, hi), bcs_fn),
                        (("nr", p, hi), rec_fn),
                        (("n", p, hi), mul_fn),
                    ]
                return items

            # --- output projection partials for pair p
            def o_items(p):
                items = []
                for qt in range(Q // 128):
                    for half in range(2):
                        n0 = 320 * half

                        def o_fn(p=p, qt=qt, n0=n0):
                            ctxT = sbuf_live["ctxT", p]
                            ps = ps_pj.tile(
                                [128, Q], F32, tag="pj", name=f"ops{p}{qt}{n0}"
                            )
                            for hi in range(2):
                                nc.tensor.matmul(
                                    ps[:, 0:320],
                                    lhsT=ctxT[:, hi, 128 * qt : 128 * (qt + 1)],
                                    rhs=wob_sb[:, 2 * p + hi, n0 : n0 + 320],
                                    start=(hi == 0),
                                    stop=(hi == 1),
                                )
                            if p == 0:
                                nc.vector.tensor_copy(
                                    out=oacc[:, qt, n0 : n0 + 320], in_=ps[:, 0:320]
                                )
                            else:
                                nc.vector.tensor_add(
                                    oacc[:, qt, n0 : n0 + 320],
                                    oacc[:, qt, n0 : n0 + 320],
                                    ps[:, 0:320],
                                )

                        items.append((("o", p, qt, half), o_fn))
                return items

            # ---- prologue: q head 0 inline, everything else queued ----
            for it in q_items(0):
                run_item(it)
            for h in range(1, H):
                work.extend(q_items(h))
            queue_kproj(0)
            for t in range(NKT):
                work.extend(v_items(0, t))
            queue_kproj(1)
            for t in range(NKT):
                work.extend(v_items(1, t))

            # ---- attention ----
            for p in range(n_pair):
                ctx2 = ps_ctx.tile([D + 1, 2, Q], F32, tag="ctx", name=f"ctx{p}")
                if p + 2 < n_pair:
                    stage_bg(p + 2)
                if p + 1 < n_pair and p + 1 not in kT_t:
                    queue_kproj(p + 1)
                if p == 2:
                    for t in range(NKT):
                        work.extend(v_items(2, t))

                pend = deque()

                def superstep(kind, ss, hi, p=p):
                    h = 2 * p + hi
                    if kind == "bg":
                        klhs = lambda par: kbg_t[p][
                            64 * par : 64 * (par + 1), hi, 128 * ss : 128 * (ss + 1)
                        ]
                    else:
                        f, c4 = ss // 4, ss % 4
                        klhs = lambda par: kT_t[p][
                            64 * par : 64 * (par + 1), hi, f, 128 * c4 : 128 * (c4 + 1)
                        ]
                    sc = ps_sc.tile(
                        [128, 2, Q], F32, tag="sc", name=f"sc{kind}{p}{ss}{hi}"
                    )
                    for par in range(2):
                        nc.tensor.matmul(
                            sc[:, par, :],
                            lhsT=klhs(par),
                            rhs=qdup[64 * par : 64 * (par + 1), h, :],
                            start=True,
                            stop=True,
                            tile_position=(64 * par, 0),
                        )
                    pr = probs_pool.tile(
                        [128, 2, Q], F16, tag="pr", name=f"pr{kind}{p}{ss}{hi}"
                    )
                    nc.scalar.activation(pr, sc, AF.Exp, scale=SCALE)
                    return pr

                def do_pv(kind, ss, hi, pr, p=p, ctx2=ctx2):
                    h = 2 * p + hi
                    g = 0 if h < 4 else (1 if h < 8 else 2)
                    for j in range(2):
                        t = 2 * ss + j
                        if kind == "bg":
                            vlhs = vbg_t[p][:, t, 65 * hi : 65 * (hi + 1)]
                        else:
                            ensure(("v", g, t))
                            vlhs = v2[:, t, h, :]
                        first = kind == "bg" and ss == 0 and j == 0
                        last = kind == "self" and ss == 15 and j == 1
                        nc.tensor.matmul(
                            ctx2[:, hi, :],
                            lhsT=vlhs,
                            rhs=pr[:, j, :],
                            start=first,
                            stop=last,
                        )

                steps = [("bg", ss, hi) for ss in range(16) for hi in range(2)]
                steps += [("self", ss, hi) for ss in range(16) for hi in range(2)]
                for si, (kind, ss, hi) in enumerate(steps):
                    if kind == "bg" and ss == 0:
                        ensure(("q", 2 * p + hi))
                    if kind == "self" and ss == 0:
                        for f in range(4):
                            ensure(("k", p, hi, f))
                    if p == 0:
                        n_dr = 0 if si < 12 else (3 if si < 32 else 2)
                    else:
                        n_dr = 2
                    pr = superstep(kind, ss, hi)
                    drain(n_dr // 2)
                    while pend:
                        do_pv(*pend.popleft())
                    drain(n_dr - n_dr // 2)
                    pend.append((kind, ss, hi, pr))
                while pend:
                    do_pv(*pend.popleft())

                # free ctx PSUM immediately; normalization runs as items
                ctxU = fin_pool.tile(
                    [D + 1, 2, Q], F16, tag="ctxU", name=f"cu{p}", bufs=2
                )
                sbuf_live["ctxU", p] = ctxU
                nc.vector.tensor_copy(out=ctxU, in_=ctx2)
                work.extend(n_items(p))
                work.extend(o_items(p))

            drain(len(work))

            # ---- final output DMA (f32 cast happens on host) ----
            nc.sync.dma_start(
                out=out.rearrange("(qt p) c -> p qt c", p=128), in_=oacc
            )
    return nc


def split_waits(nc, limit=1):
    """Hoist excess sync waits onto standalone EventSemaphore instructions."""
    cnt = 0
    for f in nc.m.functions:
        for bb in f.blocks:
            fixed = []
            for inst in bb.instructions:
                si = inst.sync_info
                if si is not None and len(si.on_wait) > limit:
                    waits = list(si.on_wait)
                    extra, keep = waits[:-limit], waits[-limit:]
                    for w in extra:
                        cnt += 1
                        ev = mybir.InstEventSemaphore(
                            name=f"I-waitsplit-{cnt}", ins=[], outs=[]
                        )
                        ev.engine = inst.engine
                        ev.sync_info = mybir.SyncInfo(on_wait=[w], on_update=[])
                        nc.register_instruction(ev)
                        fixed.append(ev)
                    si.on_wait = keep
                fixed.append(inst)
            bb.instructions[:] = fixed
    return cnt


def build_bass():
    nc = bass.Bass()
    emit(nc)
    split_waits(nc)
    return nc


def make_in_maps(hidden_states, K_bg, V_bg, Wq, Wk, Wv, Wo, bo):
    f16 = np.float16
    hidden = np.asarray(hidden_states, np.float32)[0]  # [L, C]
    hT5 = np.ascontiguousarray(hidden.T.reshape(NCC, 128, L)).astype(f16)

    def chunk_w(W):
        return np.ascontiguousarray(
            np.asarray(W, np.float32).reshape(NCC, 128, C)
        ).astype(f16)

    wq5, wk5, wv5 = chunk_w(Wq), chunk_w(Wk), chunk_w(Wv)

    WoB = np.zeros((H, D + 1, C), np.float32)
    WoB[:, :D, :] = np.asarray(Wo, np.float32).reshape(H, D, C)
    WoB[0, D, :] = np.asarray(bo, np.float32)

    # bg K: [H, 128, L/2], alpha folded, even key tiles on rows 0:64
    KbgT = np.asarray(K_bg, np.float32).transpose(0, 2, 1) * ALPHA  # [H, D, L]
    kv = KbgT.reshape(H, D, NKT, 128)
    kbgS = np.empty((H, 128, L // 2), np.float32)
    kbgS[:, 0:D, :] = kv[:, :, 0::2, :].reshape(H, D, L // 2)
    kbgS[:, D:128, :] = kv[:, :, 1::2, :].reshape(H, D, L // 2)
    kbgS = np.ascontiguousarray(kbgS).astype(f16)

    # bg V: [n_pair, 128, NKT*130], alpha folded, ones baked at cols 64/129
    Vb = (np.asarray(V_bg, np.float32) * ALPHA).reshape(H, NKT, 128, D)
    vbgS = np.ones((H // 2, 128, NKT, 130), np.float32)
    for hi in range(2):
        vbgS[:, :, :, 65 * hi : 65 * hi + D] = Vb[hi::2].transpose(0, 2, 1, 3)
    vbgS = np.ascontiguousarray(vbgS.reshape(H // 2, 128, NKT * 130)).astype(f16)

    common = {
        "hT": hT5,
        "wq": wq5,
        "wk": wk5,
        "wv": wv5,
        "wob": WoB.astype(f16),
        "kbgS": kbgS,
        "vbgS": vbgS,
    }
    return [
        dict(common, hq=np.ascontiguousarray(hT5[:, :, Q * c : Q * (c + 1)]))
        for c in range(N_CORES)
    ]


_NC_CACHE = {}


def kernel(hidden_states, K_bg, V_bg, Wq, Wk, Wv, Wo, bo):
    if "nc" not in _NC_CACHE:
        _NC_CACHE["nc"] = build_bass()
    nc = _NC_CACHE["nc"]
    in_maps = make_in_maps(hidden_states, K_bg, V_bg, Wq, Wk, Wv, Wo, bo)
    from concourse import bass2jax

    results = bass2jax.run_bass_via_pjrt(nc, in_maps, n_cores=N_CORES)
    out = np.concatenate(
        [np.asarray(results[c]["out"], np.float32) for c in range(N_CORES)], axis=0
    )
    return out.reshape(B, L, C)


# revision 17
# speedup vs baseline: 1.3520x; 1.0109x over previous
"""CARC attention processor kernel for 8 Trainium2 NeuronCores — v3.

Reference computation (B=1, L=4096, C=640, H=10, D=64):
    q/k/v = hidden @ Wq/Wk/Wv, split into 10 heads of 64
    k_cat = [k, 0.42*K_bg], v_cat = [v, 0.42*V_bg]   (key length 8192)
    out   = softmax(q k_cat^T / 8) v_cat, heads merged, @ Wo + bo

Sharding: queries split 512 per core; every core computes all 10 heads for
its queries (k/v projections replicated per core).

The ScalarE exp (41.9M elements/core at ~(N+352)/1.2 ns per ACTIVATE,
N=1024) is the hard floor (~356us); the kernel keeps ScalarE saturated:
  - all-fp16 data path, host pre-arranged inputs, descriptor-friendly DMA.
  - same-head QK pairing (q duplicated onto both partition halves via
    col-tiled projection; kT split even/odd key tiles across partition
    halves) so one [128,2,512] PSUM tile = 2 key tiles of one head ->
    exp N=1024, double-buffered in 4 banks; ctx 2 banks; proj 2 banks.
  - every non-attention PE task (q/k/v projections, softmax-denominator
    broadcast, normalization, output projection) is a queue of micro work
    items (<=2 matmul-chunks each) drained between supersteps, with
    deadline ensure()s — so the PE FIFO never parks a long fill in front
    of the next score matmul and the HAM clock gate stays warm.
  - v-projection is split by head-group (0-3 / 4-7 / 8-9) so its deadline
    spreads across pairs instead of all landing in pair 0.
  - normalization: ctx leaves PSUM immediately (one DVE copy); the
    denominator broadcast / reciprocal_approx_fast / multiply run as
    deferred items inside the next pair.
  - output projection accumulates per-pair partial products into an SBUF
    fp16 accumulator (items), so the epilogue is just the last pair's
    items + one DMA; the f16->f32 output cast happens on the host.
"""

from collections import deque

import numpy as np

import concourse.bass as bass
import concourse.mybir as mybir
import concourse.tile as tile

F32 = mybir.dt.float32
F16 = mybir.dt.float16
AF = mybir.ActivationFunctionType

B, L, C = 1, 4096, 640
H, D = 10, 64
ALPHA = 0.42
N_CORES = 8
SCALE = 1.0 / np.sqrt(D)  # 0.125
Q = L // N_CORES  # 512
NKT = L // 128  # 32 key tiles per source
NCC = C // 128  # 5 contraction chunks
VGROUPS = ((0, 4), (4, 8), (8, 10))  # v-projection head groups


def emit(nc: bass.Bass):
    n_pair = H // 2

    hT = nc.declare_dram_parameter("hT", [NCC, 128, L], F16, isOutput=False)
    hq = nc.declare_dram_parameter("hq", [NCC, 128, Q], F16, isOutput=False)
    wq = nc.declare_dram_parameter("wq", [NCC, 128, C], F16, isOutput=False)
    wk = nc.declare_dram_parameter("wk", [NCC, 128, C], F16, isOutput=False)
    wv = nc.declare_dram_parameter("wv", [NCC, 128, C], F16, isOutput=False)
    wob = nc.declare_dram_parameter("wob", [H, D + 1, C], F16, isOutput=False)
    kbgS = nc.declare_dram_parameter("kbgS", [H, 128, L // 2], F16, isOutput=False)
    vbgS = nc.declare_dram_parameter(
        "vbgS", [n_pair, 128, NKT * 130], F16, isOutput=False
    )
    out = nc.declare_dram_parameter("out", [Q, C], F16, isOutput=True)

    with tile.TileContext(nc) as tc:
        with (
            tc.tile_pool(name="singles", bufs=1) as singles,
            tc.tile_pool(name="kbgp", bufs=2) as kbgp,
            tc.tile_pool(name="vbgp", bufs=2) as vbgp,
            tc.tile_pool(name="ktp", bufs=2) as ktp,
            tc.tile_pool(name="probs", bufs=3) as probs_pool,
            tc.tile_pool(name="fin", bufs=2) as fin_pool,
            tc.tile_pool(name="ps_sc", bufs=2, space="PSUM") as ps_sc,
            tc.tile_pool(name="ps_ctx", bufs=1, space="PSUM") as ps_ctx,
            tc.tile_pool(name="ps_pj", bufs=2, space="PSUM") as ps_pj,
        ):
            # ---- persistent SBUF ----
            hT_sb = singles.tile([128, NCC, L], F16, tag="hT")
            hq_sb = singles.tile([128, NCC, Q], F16, tag="hq")
            wq_sb = singles.tile([128, NCC, C], F16, tag="wq")
            wk_sb = singles.tile([128, NCC, C], F16, tag="wk")
            wv_sb = singles.tile([128, NCC, C], F16, tag="wv")
            wob_sb = singles.tile([D + 1, H, C], F16, tag="wob")
            qdup = singles.tile([128, H, Q], F16, tag="qdup")
            v2 = singles.tile([128, NKT, H, D + 1], F16, tag="v2")
            oacc = singles.tile([128, Q // 128, C], F16, tag="oacc")
            ones64 = singles.tile([D + 1, D], F16, tag="ones64")
            nc.vector.memset(ones64, 1.0)
            nc.vector.memset(v2[:, :, :, D : D + 1], 1.0)

            # ---- input DMAs ordered by first-use deadline ----
            nc.sync.dma_start(out=hq_sb, in_=hq.rearrange("i p n -> p i n"))
            nc.sync.dma_start(out=wq_sb, in_=wq.rearrange("i p n -> p i n"))
            kbg_t = {}
            vbg_t = {}

            def stage_bg(p):
                kbg_t[p] = kbgp.tile([128, 2, L // 2], F16, tag="kbg", name=f"kbg{p}")
                for hi in range(2):
                    nc.sync.dma_start(
                        out=kbg_t[p][:, hi, :], in_=kbgS[2 * p + hi, :, :]
                    )
                vbg_t[p] = vbgp.tile([128, NKT, 130], F16, tag="vbg", name=f"vbg{p}")
                nc.sync.dma_start(
                    out=vbg_t[p].rearrange("p t c -> p (t c)"), in_=vbgS[p]
                )

            stage_bg(0)
            nc.sync.dma_start(out=wk_sb, in_=wk.rearrange("i p n -> p i n"))
            nc.sync.dma_start(out=hT_sb, in_=hT.rearrange("i p n -> p i n"))
            nc.sync.dma_start(out=wv_sb, in_=wv.rearrange("i p n -> p i n"))
            nc.sync.dma_start(out=wob_sb, in_=wob.rearrange("h p n -> p h n"))
            stage_bg(1)

            # ---------- deferred micro work items ----------
            kT_t = {}
            psum_live = {}  # fill key -> live psum tile
            sbuf_live = {}  # ctxU / rec / ctxT tiles per pair
            work = deque()
            done = set()

            def run_item(item):
                key, fn = item
                fn()
                done.add(key)

            def drain(n=1):
                for _ in range(n):
                    if work:
                        run_item(work.popleft())

            def ensure(key):
                while key not in done:
                    assert work, f"work item {key} never queued"
                    run_item(work.popleft())

            # --- q projection: head h duplicated via col-tiled matmuls
            def q_items(h):
                def fill(i0, i1, h=h):
                    if i0 == 0:
                        psum_live["q", h] = ps_pj.tile(
                            [128, Q], F32, tag="pj", name=f"qps{h}"
                        )
                    ps = psum_live["q", h]
                    for i in range(i0, i1):
                        for par in range(2):
                            nc.tensor.matmul(
                                ps[64 * par : 64 * (par + 1), :],
                                lhsT=wq_sb[:, i, 64 * h : 64 * (h + 1)],
                                rhs=hq_sb[:, i, :],
                                start=(i == 0),
                                stop=(i == NCC - 1),
                                tile_position=(0, 64 * par),
                                skip_group_check=True,
                            )

                def copy(h=h):
                    nc.vector.tensor_copy(
                        out=qdup[:, h, :], in_=psum_live.pop(("q", h))
                    )

                return [
                    (("qf", h, 0), lambda h=h: fill(0, 3)),
                    (("qf", h, 1), lambda h=h: fill(3, NCC)),
                    (("q", h), copy),
                ]

            # --- k projection: pair p, head-in-pair hi, fill f (1024 keys,
            # even tiles -> partitions 0:64, odd -> 64:128)
            def k_items(p, hi, f):
                h = 2 * p + hi

                def chunk(i0, i1, p=p, hi=hi, f=f, h=h):
                    if i0 == 0:
                        psum_live["k", p, hi, f] = ps_pj.tile(
                            [128, Q], F32, tag="pj", name=f"kps{p}{hi}{f}"
                        )
                    ps = psum_live["k", p, hi, f]
                    hT_blk = hT_sb[:, :, 1024 * f : 1024 * (f + 1)].rearrange(
                        "p i (a b n) -> p i a b n", b=2, n=128
                    )
                    for i in range(i0, i1):
                        for par in range(2):
                            nc.tensor.matmul(
                                ps[64 * par : 64 * (par + 1), :],
                                lhsT=wk_sb[:, i, 64 * h : 64 * (h + 1)],
                                rhs=hT_blk[:, i, :, par, :],
                                start=(i == 0),
                                stop=(i == NCC - 1),
                                tile_position=(0, 64 * par),
                                skip_group_check=True,
                            )

                def copy(p=p, hi=hi, f=f):
                    nc.vector.tensor_copy(
                        out=kT_t[p][:, hi, f, :], in_=psum_live.pop(("k", p, hi, f))
                    )

                return [
                    (("kf", p, hi, f, 0), lambda p=p, hi=hi, f=f: chunk(0, 2)),
                    (("kf", p, hi, f, 1), lambda p=p, hi=hi, f=f: chunk(2, 4)),
                    (("kf", p, hi, f, 2), lambda p=p, hi=hi, f=f: chunk(4, NCC)),
                    (("k", p, hi, f), copy),
                ]

            def queue_kproj(p):
                kT_t[p] = ktp.tile([128, 2, 4, Q], F16, tag="kT", name=f"kT{p}")
                for hi in range(2):
                    for f in range(4):
                        work.extend(k_items(p, hi, f))

            # --- v projection for head group g, key tile t
            def v_items(g, t):
                h0, h1 = VGROUPS[g]
                nh = h1 - h0

                def chunk(i0, i1, g=g, t=t, h0=h0, nh=nh):
                    if i0 == 0:
                        psum_live["v", g, t] = ps_pj.tile(
                            [128, Q], F32, tag="pj", name=f"vps{g}_{t}"
                        )
                    ps = psum_live["v", g, t]
                    for i in range(i0, i1):
                        nc.tensor.matmul(
                            ps[:, 0 : 64 * nh],
                            lhsT=hT_sb[:, i, 128 * t : 128 * (t + 1)],
                            rhs=wv_sb[:, i, 64 * h0 : 64 * (h0 + nh)],
                            start=(i == 0),
                            stop=(i == NCC - 1),
                        )

                def copy(g=g, t=t, h0=h0, nh=nh):
                    ps = psum_live.pop(("v", g, t))
                    nc.vector.tensor_copy(
                        out=v2[:, t, h0 : h0 + nh, 0:D],
                        in_=ps[:, 0 : 64 * nh].rearrange("p (h d) -> p h d", d=64),
                    )

                return [
                    (("vf", g, t, 0), lambda g=g, t=t: chunk(0, 3)),
                    (("vf", g, t, 1), lambda g=g, t=t: chunk(3, NCC)),
                    (("v", g, t), copy),
                ]

            # --- normalization items for pair p (after ctxU copied to SBUF)
            def n_items(p):
                if p >= 2:
                    # pair p's ctxT reuses pair p-2's buffer; its readers
                    # (o-items of p-2, queued into pair p-1) must be done
                    ensure(("o", p - 2, Q // 128 - 1, 1))
                items = []
                ctxT = fin_pool.tile(
                    [D + 1, 2, Q], F16, tag="ctxT", name=f"ctxT{p}", bufs=2
                )
                sbuf_live["ctxT", p] = ctxT
                for hi in range(2):

                    def bc_fn(p=p, hi=hi):
                        ctxU = sbuf_live["ctxU", p]
                        bc = ps_pj.tile([D, Q], F32, tag="pj", name=f"bc{p}{hi}")
                        psum_live["bc", p, hi] = bc
                        nc.tensor.matmul(
                            bc,
                            lhsT=ones64[D : D + 1, :],
                            rhs=ctxU[D : D + 1, hi, :],
                            start=True,
                            stop=True,
                            tile_position=(D, 0),
                        )

                    def bcs_fn(p=p, hi=hi):
                        bcs = fin_pool.tile(
                            [D, Q], F32, tag="bcs", name=f"bcs{p}{hi}", bufs=2
                        )
                        sbuf_live["bcs", p, hi] = bcs
                        nc.vector.tensor_copy(
                            out=bcs, in_=psum_live.pop(("bc", p, hi))
                        )

                    def rec_fn(p=p, hi=hi):
                        rec = fin_pool.tile(
                            [D, Q], F32, tag="rec", name=f"rec{p}{hi}", bufs=2
                        )
                        sbuf_live["rec", p, hi] = rec
                        nc.vector.reciprocal(rec, sbuf_live.pop(("bcs", p, hi)))

                    def mul_fn(p=p, hi=hi):
                        ctxU = sbuf_live["ctxU", p]
                        ctxT_ = sbuf_live["ctxT", p]
                        nc.vector.tensor_mul(
                            ctxT_[0:D, hi, :],
                            ctxU[0:D, hi, :],
                            sbuf_live.pop(("rec", p, hi)),
                        )
                        nc.vector.memset(ctxT_[D : D + 1, hi, :], 1.0)

                    items += [
                        (("nb", p, hi), bc_fn),
                        (("nc", p, hi), bcs_fn),
                        (("nr", p, hi), rec_fn),
                        (("n", p, hi), mul_fn),
                    ]
                return items

            # --- output projection partials for pair p
            def o_items(p):
                items = []
                for qt in range(Q // 128):
                    for half in range(2):
                        n0 = 320 * half

                        def o_fn(p=p, qt=qt, n0=n0):
                            ctxT = sbuf_live["ctxT", p]
                            ps = ps_pj.tile(
                                [128, Q], F32, tag="pj", name=f"ops{p}{qt}{n0}"
                            )
                            for hi in range(2):
                                nc.tensor.matmul(
                                    ps[:, 0:320],
                                    lhsT=ctxT[:, hi, 128 * qt : 128 * (qt + 1)],
                                    rhs=wob_sb[:, 2 * p + hi, n0 : n0 + 320],
                                    start=(hi == 0),
                                    stop=(hi == 1),
                                )
                            if p == 0:
                                nc.vector.tensor_copy(
                                    out=oacc[:, qt, n0 : n0 + 320], in_=ps[:, 0:320]
                                )
                            else:
                                nc.vector.tensor_add(
                                    oacc[:, qt, n0 : n0 + 320],
                                    oacc[:, qt, n0 : n0 + 320],
                                    ps[:, 0:320],
                                )

                        items.append((("o", p, qt, half), o_fn))
                return items

            # ---- prologue: q head 0 inline, everything else queued ----
            for it in q_items(0):
                run_item(it)
            for h in range(1, H):
                work.extend(q_items(h))
            queue_kproj(0)
            for t in range(NKT):
                work.extend(v_items(0, t))
            queue_kproj(1)
            for t in range(NKT):
                work.extend(v_items(1, t))

            # ---- attention ----
            for p in range(n_pair):
                ctx2 = ps_ctx.tile([D + 1, 2, Q], F32, tag="ctx", name=f"ctx{p}")
                if p + 2 < n_pair:
                    stage_bg(p + 2)
                if p + 1 < n_pair and p + 1 not in kT_t:
                    queue_kproj(p + 1)
                if p == 2:
                    for t in range(NKT):
                        work.extend(v_items(2, t))

                pend = deque()

                def superstep(kind, ss, hi, p=p):
                    h = 2 * p + hi
                    if kind == "bg":
                        klhs = lambda par: kbg_t[p][
                            64 * par : 64 * (par + 1), hi, 128 * ss : 128 * (ss + 1)
                        ]
                    else:
                        f, c4 = ss // 4, ss % 4
                        klhs = lambda par: kT_t[p][
                            64 * par : 64 * (par + 1), hi, f, 128 * c4 : 128 * (c4 + 1)
                        ]
                    sc = ps_sc.tile(
                        [128, 2, Q], F32, tag="sc", name=f"sc{kind}{p}{ss}{hi}"
                    )
                    for par in range(2):
                        nc.tensor.matmul(
                            sc[:, par, :],
                            lhsT=klhs(par),
                            rhs=qdup[64 * par : 64 * (par + 1), h, :],
                            start=True,
                            stop=True,
                            tile_position=(64 * par, 0),
                        )
                    pr = probs_pool.tile(
                        [128, 2, Q], F16, tag="pr", name=f"pr{kind}{p}{ss}{hi}"
                    )
                    nc.scalar.activation(pr, sc, AF.Exp, scale=SCALE)
                    return pr

                def do_pv(kind, ss, hi, pr, p=p, ctx2=ctx2):
                    h = 2 * p + hi
                    g = 0 if h < 4 else (1 if h < 8 else 2)
                    for j in range(2):
                        t = 2 * ss + j
                        if kind == "bg":
                            vlhs = vbg_t[p][:, t, 65 * hi : 65 * (hi + 1)]
                        else:
                            ensure(("v", g, t))
                            vlhs = v2[:, t, h, :]
                        first = kind == "bg" and ss == 0 and j == 0
                        last = kind == "self" and ss == 15 and j == 1
                        nc.tensor.matmul(
                            ctx2[:, hi, :],
                            lhsT=vlhs,
                            rhs=pr[:, j, :],
                            start=first,
                            stop=last,
                        )

                steps = [("bg", ss, hi) for ss in range(16) for hi in range(2)]
                steps += [("self", ss, hi) for ss in range(16) for hi in range(2)]
                for si, (kind, ss, hi) in enumerate(steps):
                    if si == 32 and p > 0:
                        work.extend(o_items(p - 1))
                    if kind == "bg" and ss == 0:
                        ensure(("q", 2 * p + hi))
                    if kind == "self" and ss == 0:
                        for f in range(4):
                            ensure(("k", p, hi, f))
                    if p == 0:
                        n_dr = 0 if si < 12 else (3 if si < 32 else 2)
                    else:
                        n_dr = 2
                    pr = superstep(kind, ss, hi)
                    drain(n_dr // 2)
                    while pend:
                        do_pv(*pend.popleft())
                    drain(n_dr - n_dr // 2)
                    pend.append((kind, ss, hi, pr))
                while pend:
                    do_pv(*pend.popleft())

                # free ctx PSUM immediately; normalization runs as items
                ctxU = fin_pool.tile(
                    [D + 1, 2, Q], F16, tag="ctxU", name=f"cu{p}", bufs=2
                )
                sbuf_live["ctxU", p] = ctxU
                nc.vector.tensor_copy(out=ctxU, in_=ctx2)
                work.extend(n_items(p))
                if p == n_pair - 1:
                    work.extend(o_items(p))

            drain(len(work))

            # ---- final output DMA (f32 cast happens on host) ----
            nc.sync.dma_start(
                out=out.rearrange("(qt p) c -> p qt c", p=128), in_=oacc
            )
    return nc


def split_waits(nc, limit=1):
    """Hoist excess sync waits onto standalone EventSemaphore instructions."""
    cnt = 0
    for f in nc.m.functions:
        for bb in f.blocks:
            fixed = []
            for inst in bb.instructions:
                si = inst.sync_info
                if si is not None and len(si.on_wait) > limit:
                    waits = list(si.on_wait)
                    extra, keep = waits[:-limit], waits[-limit:]
                    for w in extra:
                        cnt += 1
                        ev = mybir.InstEventSemaphore(
                            name=f"I-waitsplit-{cnt}", ins=[], outs=[]
                        )
                        ev.engine = inst.engine
                        ev.sync_info = mybir.SyncInfo(on_wait=[w], on_update=[])
                        nc.register_instruction(ev)
                        fixed.append(ev)
                    si.on_wait = keep
                fixed.append(inst)
            bb.instructions[:] = fixed
    return cnt


def build_bass():
    nc = bass.Bass()
    emit(nc)
    split_waits(nc)
    return nc


def make_in_maps(hidden_states, K_bg, V_bg, Wq, Wk, Wv, Wo, bo):
    f16 = np.float16
    hidden = np.asarray(hidden_states, np.float32)[0]  # [L, C]
    hT5 = np.ascontiguousarray(hidden.T.reshape(NCC, 128, L)).astype(f16)

    def chunk_w(W):
        return np.ascontiguousarray(
            np.asarray(W, np.float32).reshape(NCC, 128, C)
        ).astype(f16)

    wq5, wk5, wv5 = chunk_w(Wq), chunk_w(Wk), chunk_w(Wv)

    WoB = np.zeros((H, D + 1, C), np.float32)
    WoB[:, :D, :] = np.asarray(Wo, np.float32).reshape(H, D, C)
    WoB[0, D, :] = np.asarray(bo, np.float32)

    # bg K: [H, 128, L/2], alpha folded, even key tiles on rows 0:64
    KbgT = np.asarray(K_bg, np.float32).transpose(0, 2, 1) * ALPHA  # [H, D, L]
    kv = KbgT.reshape(H, D, NKT, 128)
    kbgS = np.empty((H, 128, L // 2), np.float32)
    kbgS[:, 0:D, :] = kv[:, :, 0::2, :].reshape(H, D, L // 2)
    kbgS[:, D:128, :] = kv[:, :, 1::2, :].reshape(H, D, L // 2)
    kbgS = np.ascontiguousarray(kbgS).astype(f16)

    # bg V: [n_pair, 128, NKT*130], alpha folded, ones baked at cols 64/129
    Vb = (np.asarray(V_bg, np.float32) * ALPHA).reshape(H, NKT, 128, D)
    vbgS = np.ones((H // 2, 128, NKT, 130), np.float32)
    for hi in range(2):
        vbgS[:, :, :, 65 * hi : 65 * hi + D] = Vb[hi::2].transpose(0, 2, 1, 3)
    vbgS = np.ascontiguousarray(vbgS.reshape(H // 2, 128, NKT * 130)).astype(f16)

    common = {
        "hT": hT5,
        "wq": wq5,
        "wk": wk5,
        "wv": wv5,
        "wob": WoB.astype(f16),
        "kbgS": kbgS,
        "vbgS": vbgS,
    }
    return [
        dict(common, hq=np.ascontiguousarray(hT5[:, :, Q * c : Q * (c + 1)]))
        for c in range(N_CORES)
    ]


_NC_CACHE = {}


def kernel(hidden_states, K_bg, V_bg, Wq, Wk, Wv, Wo, bo):
    if "nc" not in _NC_CACHE:
        _NC_CACHE["nc"] = build_bass()
    nc = _NC_CACHE["nc"]
    in_maps = make_in_maps(hidden_states, K_bg, V_bg, Wq, Wk, Wv, Wo, bo)
    from concourse import bass2jax

    results = bass2jax.run_bass_via_pjrt(nc, in_maps, n_cores=N_CORES)
    out = np.concatenate(
        [np.asarray(results[c]["out"], np.float32) for c in range(N_CORES)], axis=0
    )
    return out.reshape(B, L, C)


# revision 18
# speedup vs baseline: 1.3700x; 1.0133x over previous
"""CARC attention processor kernel for 8 Trainium2 NeuronCores — v3.

Reference computation (B=1, L=4096, C=640, H=10, D=64):
    q/k/v = hidden @ Wq/Wk/Wv, split into 10 heads of 64
    k_cat = [k, 0.42*K_bg], v_cat = [v, 0.42*V_bg]   (key length 8192)
    out   = softmax(q k_cat^T / 8) v_cat, heads merged, @ Wo + bo

Sharding: queries split 512 per core; every core computes all 10 heads for
its queries (k/v projections replicated per core).

The ScalarE exp (41.9M elements/core at ~(N+352)/1.2 ns per ACTIVATE,
N=1024) is the hard floor (~356us); the kernel keeps ScalarE saturated:
  - all-fp16 data path, host pre-arranged inputs, descriptor-friendly DMA.
  - same-head QK pairing (q duplicated onto both partition halves via
    col-tiled projection; kT split even/odd key tiles across partition
    halves) so one [128,2,512] PSUM tile = 2 key tiles of one head ->
    exp N=1024, double-buffered in 4 banks; ctx 2 banks; proj 2 banks.
  - every non-attention PE task (q/k/v projections, softmax-denominator
    broadcast, normalization, output projection) is a queue of micro work
    items (<=2 matmul-chunks each) drained between supersteps, with
    deadline ensure()s — so the PE FIFO never parks a long fill in front
    of the next score matmul and the HAM clock gate stays warm.
  - v-projection is split by head-group (0-3 / 4-7 / 8-9) so its deadline
    spreads across pairs instead of all landing in pair 0.
  - normalization: ctx leaves PSUM immediately (one DVE copy); the
    denominator broadcast / reciprocal_approx_fast / multiply run as
    deferred items inside the next pair.
  - output projection accumulates per-pair partial products into an SBUF
    fp16 accumulator (items), so the epilogue is just the last pair's
    items + one DMA; the f16->f32 output cast happens on the host.
"""

from collections import deque

import numpy as np

import concourse.bass as bass
import concourse.mybir as mybir
import concourse.tile as tile

F32 = mybir.dt.float32
F16 = mybir.dt.float16
AF = mybir.ActivationFunctionType

B, L, C = 1, 4096, 640
H, D = 10, 64
ALPHA = 0.42
N_CORES = 8
SCALE = 1.0 / np.sqrt(D)  # 0.125
Q = L // N_CORES  # 512
NKT = L // 128  # 32 key tiles per source
NCC = C // 128  # 5 contraction chunks
VGROUPS = ((0, 4), (4, 8), (8, 10))  # v-projection head groups


def emit(nc: bass.Bass):
    n_pair = H // 2

    hT = nc.declare_dram_parameter("hT", [NCC, 128, L], F16, isOutput=False)
    hq = nc.declare_dram_parameter("hq", [NCC, 128, Q], F16, isOutput=False)
    wq = nc.declare_dram_parameter("wq", [NCC, 128, C], F16, isOutput=False)
    wk = nc.declare_dram_parameter("wk", [NCC, 128, C], F16, isOutput=False)
    wv = nc.declare_dram_parameter("wv", [NCC, 128, C], F16, isOutput=False)
    wob = nc.declare_dram_parameter("wob", [H, D + 1, C], F16, isOutput=False)
    kbgS = nc.declare_dram_parameter("kbgS", [H, 128, L // 2], F16, isOutput=False)
    vbgS = nc.declare_dram_parameter(
        "vbgS", [n_pair, 128, NKT * 130], F16, isOutput=False
    )
    out = nc.declare_dram_parameter("out", [Q, C], F16, isOutput=True)

    with tile.TileContext(nc) as tc:
        with (
            tc.tile_pool(name="singles", bufs=1) as singles,
            tc.tile_pool(name="kbgp", bufs=2) as kbgp,
            tc.tile_pool(name="vbgp", bufs=2) as vbgp,
            tc.tile_pool(name="ktp", bufs=2) as ktp,
            tc.tile_pool(name="probs", bufs=3) as probs_pool,
            tc.tile_pool(name="fin", bufs=2) as fin_pool,
            tc.tile_pool(name="ps_sc", bufs=2, space="PSUM") as ps_sc,
            tc.tile_pool(name="ps_ctx", bufs=1, space="PSUM") as ps_ctx,
            tc.tile_pool(name="ps_pj", bufs=2, space="PSUM") as ps_pj,
        ):
            # ---- persistent SBUF ----
            hT_sb = singles.tile([128, NCC, L], F16, tag="hT")
            hq_sb = singles.tile([128, NCC, Q], F16, tag="hq")
            wq_sb = singles.tile([128, NCC, C], F16, tag="wq")
            wk_sb = singles.tile([128, NCC, C], F16, tag="wk")
            wv_sb = singles.tile([128, NCC, C], F16, tag="wv")
            wob_sb = singles.tile([D + 1, H, C], F16, tag="wob")
            qdup = singles.tile([128, H, Q], F16, tag="qdup")
            v2 = singles.tile([128, NKT, H, D + 1], F16, tag="v2")
            oacc = singles.tile([128, Q // 128, C], F16, tag="oacc")
            ones64 = singles.tile([D + 1, D], F16, tag="ones64")
            nc.vector.memset(ones64, 1.0)
            nc.vector.memset(v2[:, :, :, D : D + 1], 1.0)

            # ---- input DMAs ordered by first-use deadline ----
            nc.sync.dma_start(out=hq_sb, in_=hq.rearrange("i p n -> p i n"))
            nc.sync.dma_start(out=wq_sb, in_=wq.rearrange("i p n -> p i n"))
            kbg_t = {}
            vbg_t = {}

            def stage_bg(p):
                kbg_t[p] = kbgp.tile([128, 2, L // 2], F16, tag="kbg", name=f"kbg{p}")
                for hi in range(2):
                    nc.sync.dma_start(
                        out=kbg_t[p][:, hi, :], in_=kbgS[2 * p + hi, :, :]
                    )
                vbg_t[p] = vbgp.tile([128, NKT, 130], F16, tag="vbg", name=f"vbg{p}")
                nc.sync.dma_start(
                    out=vbg_t[p].rearrange("p t c -> p (t c)"), in_=vbgS[p]
                )

            stage_bg(0)
            nc.sync.dma_start(out=wk_sb, in_=wk.rearrange("i p n -> p i n"))
            nc.sync.dma_start(out=hT_sb, in_=hT.rearrange("i p n -> p i n"))
            nc.sync.dma_start(out=wv_sb, in_=wv.rearrange("i p n -> p i n"))
            nc.sync.dma_start(out=wob_sb, in_=wob.rearrange("h p n -> p h n"))
            stage_bg(1)

            # ---------- deferred micro work items ----------
            kT_t = {}
            psum_live = {}  # fill key -> live psum tile
            sbuf_live = {}  # ctxU / rec / ctxT tiles per pair
            work = deque()
            done = set()

            def run_item(item):
                key, fn = item
                fn()
                done.add(key)

            def drain(n=1):
                for _ in range(n):
                    if work:
                        run_item(work.popleft())

            def ensure(key):
                while key not in done:
                    assert work, f"work item {key} never queued"
                    run_item(work.popleft())

            # --- q projection: head h duplicated via col-tiled matmuls
            def q_items(h):
                def fill(i0, i1, h=h):
                    if i0 == 0:
                        psum_live["q", h] = ps_pj.tile(
                            [128, Q], F32, tag="pj", name=f"qps{h}"
                        )
                    ps = psum_live["q", h]
                    for i in range(i0, i1):
                        for par in range(2):
                            nc.tensor.matmul(
                                ps[64 * par : 64 * (par + 1), :],
                                lhsT=wq_sb[:, i, 64 * h : 64 * (h + 1)],
                                rhs=hq_sb[:, i, :],
                                start=(i == 0),
                                stop=(i == NCC - 1),
                                tile_position=(0, 64 * par),
                                skip_group_check=True,
                            )

                def copy(h=h):
                    nc.vector.tensor_copy(
                        out=qdup[:, h, :], in_=psum_live.pop(("q", h))
                    )

                return [
                    (("qf", h, 0), lambda h=h: fill(0, 3)),
                    (("qf", h, 1), lambda h=h: fill(3, NCC)),
                    (("q", h), copy),
                ]

            # --- k projection: pair p, head-in-pair hi, fill f (1024 keys,
            # even tiles -> partitions 0:64, odd -> 64:128)
            def k_items(p, hi, f):
                h = 2 * p + hi

                def chunk(i0, i1, p=p, hi=hi, f=f, h=h):
                    if i0 == 0:
                        psum_live["k", p, hi, f] = ps_pj.tile(
                            [128, Q], F32, tag="pj", name=f"kps{p}{hi}{f}"
                        )
                    ps = psum_live["k", p, hi, f]
                    hT_blk = hT_sb[:, :, 1024 * f : 1024 * (f + 1)].rearrange(
                        "p i (a b n) -> p i a b n", b=2, n=128
                    )
                    for i in range(i0, i1):
                        for par in range(2):
                            nc.tensor.matmul(
                                ps[64 * par : 64 * (par + 1), :],
                                lhsT=wk_sb[:, i, 64 * h : 64 * (h + 1)],
                                rhs=hT_blk[:, i, :, par, :],
                                start=(i == 0),
                                stop=(i == NCC - 1),
                                tile_position=(0, 64 * par),
                                skip_group_check=True,
                            )

                def copy(p=p, hi=hi, f=f):
                    nc.vector.tensor_copy(
                        out=kT_t[p][:, hi, f, :], in_=psum_live.pop(("k", p, hi, f))
                    )

                return [
                    (("kf", p, hi, f, 0), lambda p=p, hi=hi, f=f: chunk(0, 2)),
                    (("kf", p, hi, f, 1), lambda p=p, hi=hi, f=f: chunk(2, 4)),
                    (("kf", p, hi, f, 2), lambda p=p, hi=hi, f=f: chunk(4, NCC)),
                    (("k", p, hi, f), copy),
                ]

            def queue_kproj(p):
                kT_t[p] = ktp.tile([128, 2, 4, Q], F16, tag="kT", name=f"kT{p}")
                for hi in range(2):
                    for f in range(4):
                        work.extend(k_items(p, hi, f))

            # --- v projection for head group g, key tile t
            def v_items(g, t):
                h0, h1 = VGROUPS[g]
                nh = h1 - h0

                def chunk(i0, i1, g=g, t=t, h0=h0, nh=nh):
                    if i0 == 0:
                        psum_live["v", g, t] = ps_pj.tile(
                            [128, Q], F32, tag="pj", name=f"vps{g}_{t}"
                        )
                    ps = psum_live["v", g, t]
                    for i in range(i0, i1):
                        nc.tensor.matmul(
                            ps[:, 0 : 64 * nh],
                            lhsT=hT_sb[:, i, 128 * t : 128 * (t + 1)],
                            rhs=wv_sb[:, i, 64 * h0 : 64 * (h0 + nh)],
                            start=(i == 0),
                            stop=(i == NCC - 1),
                        )

                def copy(g=g, t=t, h0=h0, nh=nh):
                    ps = psum_live.pop(("v", g, t))
                    nc.vector.tensor_copy(
                        out=v2[:, t, h0 : h0 + nh, 0:D],
                        in_=ps[:, 0 : 64 * nh].rearrange("p (h d) -> p h d", d=64),
                    )

                return [
                    (("vf", g, t, 0), lambda g=g, t=t: chunk(0, 3)),
                    (("vf", g, t, 1), lambda g=g, t=t: chunk(3, NCC)),
                    (("v", g, t), copy),
                ]

            # --- normalization items for pair p (after ctxU copied to SBUF)
            def n_items(p):
                if p >= 2:
                    # pair p's ctxT reuses pair p-2's buffer; its readers
                    # (o-items of p-2, queued into pair p-1) must be done
                    ensure(("o", p - 2, Q // 128 - 1, 1))
                items = []
                ctxT = fin_pool.tile(
                    [D + 1, 2, Q], F16, tag="ctxT", name=f"ctxT{p}", bufs=2
                )
                sbuf_live["ctxT", p] = ctxT
                for hi in range(2):

                    def bc_fn(p=p, hi=hi):
                        ctxU = sbuf_live["ctxU", p]
                        bc = ps_pj.tile([D, Q], F32, tag="pj", name=f"bc{p}{hi}")
                        psum_live["bc", p, hi] = bc
                        nc.tensor.matmul(
                            bc,
                            lhsT=ones64[D : D + 1, :],
                            rhs=ctxU[D : D + 1, hi, :],
                            start=True,
                            stop=True,
                            tile_position=(D, 0),
                        )

                    def bcs_fn(p=p, hi=hi):
                        bcs = fin_pool.tile(
                            [D, Q], F32, tag="bcs", name=f"bcs{p}{hi}", bufs=2
                        )
                        sbuf_live["bcs", p, hi] = bcs
                        nc.vector.tensor_copy(
                            out=bcs, in_=psum_live.pop(("bc", p, hi))
                        )

                    def rec_fn(p=p, hi=hi):
                        rec = fin_pool.tile(
                            [D, Q], F32, tag="rec", name=f"rec{p}{hi}", bufs=2
                        )
                        sbuf_live["rec", p, hi] = rec
                        nc.vector.reciprocal(rec, sbuf_live.pop(("bcs", p, hi)))

                    def mul_fn(p=p, hi=hi):
                        ctxU = sbuf_live["ctxU", p]
                        ctxT_ = sbuf_live["ctxT", p]
                        nc.vector.tensor_mul(
                            ctxT_[0:D, hi, :],
                            ctxU[0:D, hi, :],
                            sbuf_live.pop(("rec", p, hi)),
                        )
                        nc.vector.memset(ctxT_[D : D + 1, hi, :], 1.0)

                    items += [
                        (("nb", p, hi), bc_fn),
                        (("nc", p, hi), bcs_fn),
                        (("nr", p, hi), rec_fn),
                        (("n", p, hi), mul_fn),
                    ]
                return items

            # --- output projection partials for pair p
            def o_items(p):
                items = []
                for qt in range(Q // 128):
                    for half in range(2):
                        n0 = 320 * half

                        def o_fn(p=p, qt=qt, n0=n0):
                            ctxT = sbuf_live["ctxT", p]
                            ps = ps_pj.tile(
                                [128, Q], F32, tag="pj", name=f"ops{p}{qt}{n0}"
                            )
                            for hi in range(2):
                                nc.tensor.matmul(
                                    ps[:, 0:320],
                                    lhsT=ctxT[:, hi, 128 * qt : 128 * (qt + 1)],
                                    rhs=wob_sb[:, 2 * p + hi, n0 : n0 + 320],
                                    start=(hi == 0),
                                    stop=(hi == 1),
                                )
                            if p == 0:
                                nc.vector.tensor_copy(
                                    out=oacc[:, qt, n0 : n0 + 320], in_=ps[:, 0:320]
                                )
                            else:
                                nc.vector.tensor_add(
                                    oacc[:, qt, n0 : n0 + 320],
                                    oacc[:, qt, n0 : n0 + 320],
                                    ps[:, 0:320],
                                )

                        items.append((("o", p, qt, half), o_fn))
                return items

            # ---- prologue: q head 0 inline, everything else queued ----
            for it in q_items(0):
                run_item(it)
            for h in range(1, H):
                work.extend(q_items(h))
            queue_kproj(0)
            for t in range(NKT):
                work.extend(v_items(0, t))
            queue_kproj(1)
            for t in range(NKT):
                work.extend(v_items(1, t))

            # ---- attention ----
            for p in range(n_pair):
                ctx2 = ps_ctx.tile([D + 1, 2, Q], F32, tag="ctx", name=f"ctx{p}")
                if p + 2 < n_pair:
                    stage_bg(p + 2)
                if p + 1 < n_pair and p + 1 not in kT_t:
                    queue_kproj(p + 1)
                if p == 2:
                    for t in range(NKT):
                        work.extend(v_items(2, t))

                pend = deque()

                def superstep(kind, ss, hi, p=p):
                    h = 2 * p + hi
                    if kind == "bg":
                        klhs = lambda par: kbg_t[p][
                            64 * par : 64 * (par + 1), hi, 128 * ss : 128 * (ss + 1)
                        ]
                    else:
                        f, c4 = ss // 4, ss % 4
                        klhs = lambda par: kT_t[p][
                            64 * par : 64 * (par + 1), hi, f, 128 * c4 : 128 * (c4 + 1)
                        ]
                    sc = ps_sc.tile(
                        [128, 2, Q], F32, tag="sc", name=f"sc{kind}{p}{ss}{hi}"
                    )
                    # 4 concurrent 64x64 quadrant matmuls: row = even/odd key
                    # tile (kT layout), col = lo/hi key half; each gets its
                    # own XBUS stream so both tiles finish in ~512 cycles.
                    for par in range(2):
                        kl = klhs(par)
                        for co in range(2):
                            nc.tensor.matmul(
                                sc[64 * co : 64 * (co + 1), par, :],
                                lhsT=kl[:, 64 * co : 64 * (co + 1)],
                                rhs=qdup[64 * par : 64 * (par + 1), h, :],
                                start=True,
                                stop=True,
                                tile_position=(64 * par, 64 * co),
                                skip_group_check=True,
                            )
                    pr = probs_pool.tile(
                        [128, 2, Q], F16, tag="pr", name=f"pr{kind}{p}{ss}{hi}"
                    )
                    nc.scalar.activation(pr, sc, AF.Exp, scale=SCALE)
                    return pr

                def do_pv(kind, ss, hi, pr, p=p, ctx2=ctx2):
                    h = 2 * p + hi
                    g = 0 if h < 4 else (1 if h < 8 else 2)
                    for j in range(2):
                        t = 2 * ss + j
                        if kind == "bg":
                            vlhs = vbg_t[p][:, t, 65 * hi : 65 * (hi + 1)]
                        else:
                            ensure(("v", g, t))
                            vlhs = v2[:, t, h, :]
                        first = kind == "bg" and ss == 0 and j == 0
                        last = kind == "self" and ss == 15 and j == 1
                        nc.tensor.matmul(
                            ctx2[:, hi, :],
                            lhsT=vlhs,
                            rhs=pr[:, j, :],
                            start=first,
                            stop=last,
                        )

                steps = [("bg", ss, hi) for ss in range(16) for hi in range(2)]
                steps += [("self", ss, hi) for ss in range(16) for hi in range(2)]
                for si, (kind, ss, hi) in enumerate(steps):
                    if si == 32 and p > 0:
                        work.extend(o_items(p - 1))
                    if kind == "bg" and ss == 0:
                        ensure(("q", 2 * p + hi))
                    if kind == "self" and ss == 0:
                        for f in range(4):
                            ensure(("k", p, hi, f))
                    if p == 0:
                        n_dr = 0 if si < 12 else (3 if si < 32 else 2)
                    else:
                        n_dr = 2
                    pr = superstep(kind, ss, hi)
                    drain(n_dr // 2)
                    while pend:
                        do_pv(*pend.popleft())
                    drain(n_dr - n_dr // 2)
                    pend.append((kind, ss, hi, pr))
                while pend:
                    do_pv(*pend.popleft())

                # free ctx PSUM immediately; normalization runs as items
                ctxU = fin_pool.tile(
                    [D + 1, 2, Q], F16, tag="ctxU", name=f"cu{p}", bufs=2
                )
                sbuf_live["ctxU", p] = ctxU
                nc.vector.tensor_copy(out=ctxU, in_=ctx2)
                work.extend(n_items(p))
                if p == n_pair - 1:
                    work.extend(o_items(p))

            drain(len(work))

            # ---- final output DMA (f32 cast happens on host) ----
            nc.sync.dma_start(
                out=out.rearrange("(qt p) c -> p qt c", p=128), in_=oacc
            )
    return nc


def split_waits(nc, limit=1):
    """Hoist excess sync waits onto standalone EventSemaphore instructions."""
    cnt = 0
    for f in nc.m.functions:
        for bb in f.blocks:
            fixed = []
            for inst in bb.instructions:
                si = inst.sync_info
                if si is not None and len(si.on_wait) > limit:
                    waits = list(si.on_wait)
                    extra, keep = waits[:-limit], waits[-limit:]
                    for w in extra:
                        cnt += 1
                        ev = mybir.InstEventSemaphore(
                            name=f"I-waitsplit-{cnt}", ins=[], outs=[]
                        )
                        ev.engine = inst.engine
                        ev.sync_info = mybir.SyncInfo(on_wait=[w], on_update=[])
                        nc.register_instruction(ev)
                        fixed.append(ev)
                    si.on_wait = keep
                fixed.append(inst)
            bb.instructions[:] = fixed
    return cnt


def build_bass():
    nc = bass.Bass()
    emit(nc)
    split_waits(nc)
    return nc


def make_in_maps(hidden_states, K_bg, V_bg, Wq, Wk, Wv, Wo, bo):
    f16 = np.float16
    hidden = np.asarray(hidden_states, np.float32)[0]  # [L, C]
    hT5 = np.ascontiguousarray(hidden.T.reshape(NCC, 128, L)).astype(f16)

    def chunk_w(W):
        return np.ascontiguousarray(
            np.asarray(W, np.float32).reshape(NCC, 128, C)
        ).astype(f16)

    wq5, wk5, wv5 = chunk_w(Wq), chunk_w(Wk), chunk_w(Wv)

    WoB = np.zeros((H, D + 1, C), np.float32)
    WoB[:, :D, :] = np.asarray(Wo, np.float32).reshape(H, D, C)
    WoB[0, D, :] = np.asarray(bo, np.float32)

    # bg K: [H, 128, L/2], alpha folded, even key tiles on rows 0:64
    KbgT = np.asarray(K_bg, np.float32).transpose(0, 2, 1) * ALPHA  # [H, D, L]
    kv = KbgT.reshape(H, D, NKT, 128)
    kbgS = np.empty((H, 128, L // 2), np.float32)
    kbgS[:, 0:D, :] = kv[:, :, 0::2, :].reshape(H, D, L // 2)
    kbgS[:, D:128, :] = kv[:, :, 1::2, :].reshape(H, D, L // 2)
    kbgS = np.ascontiguousarray(kbgS).astype(f16)

    # bg V: [n_pair, 128, NKT*130], alpha folded, ones baked at cols 64/129
    Vb = (np.asarray(V_bg, np.float32) * ALPHA).reshape(H, NKT, 128, D)
    vbgS = np.ones((H // 2, 128, NKT, 130), np.float32)
    for hi in range(2):
        vbgS[:, :, :, 65 * hi : 65 * hi + D] = Vb[hi::2].transpose(0, 2, 1, 3)
    vbgS = np.ascontiguousarray(vbgS.reshape(H // 2, 128, NKT * 130)).astype(f16)

    common = {
        "hT": hT5,
        "wq": wq5,
        "wk": wk5,
        "wv": wv5,
        "wob": WoB.astype(f16),
        "kbgS": kbgS,
        "vbgS": vbgS,
    }
    return [
        dict(common, hq=np.ascontiguousarray(hT5[:, :, Q * c : Q * (c + 1)]))
        for c in range(N_CORES)
    ]


_NC_CACHE = {}


def kernel(hidden_states, K_bg, V_bg, Wq, Wk, Wv, Wo, bo):
    if "nc" not in _NC_CACHE:
        _NC_CACHE["nc"] = build_bass()
    nc = _NC_CACHE["nc"]
    in_maps = make_in_maps(hidden_states, K_bg, V_bg, Wq, Wk, Wv, Wo, bo)
    from concourse import bass2jax

    results = bass2jax.run_bass_via_pjrt(nc, in_maps, n_cores=N_CORES)
    out = np.concatenate(
        [np.asarray(results[c]["out"], np.float32) for c in range(N_CORES)], axis=0
    )
    return out.reshape(B, L, C)
